# revision 34
# baseline (speedup 1.0000x reference)
"""GAT (2-layer, 4-head) forward on 8 Trainium2 NeuronCores (Bass/Tile).

Slot-aligned round design:
Destination nodes are dealt to 8 cores (degree-balanced snake deal) and,
within each core, lex-sorted by (deg, lo1-fixed, hi1-fixed, lo2-fixed)
descending into blocks of 128 slots. Edge processing is round-based:
round r of block b holds, at partition s, the r-th incoming edge of the
node at slot s (sentinel-row pads whose a_s = -1e4 make exp() == 0). The
one-hot scatter matrices of the chunked design disappear: each round
accumulates into the block PSUM through an identity matmul, and a_d lives
in SBUF per-block tiles aligned to partitions — no per-edge a_d gather at
all. Rounds split into lo/hi passes by int16 gather-index reach; rows
[0, DUP) are duplicated into the spare hi index window so ~30% of edges
can be assigned to either pass, and a per-block solver balances per-slot
lo/hi counts to minimize total rounds. Layer-1 table rows are fp16
[4x(64 h) | a_s] at 768B pitch, rotated own-first; xT ships fp8. Layer-2
rows are fp8 [4x(32 h2) | a_s2 fp16] packed to 136B for a pipelined
AllGather in 8-block groups (the int16 boundary falls on a group
boundary; the last group is one block so the tail collective is tiny),
then expanded locally to 256B-pitch gather rows. Those expansion copies
are the only users of the sync DMA queue during the edge phases, so their
head-of-line wait on the collectives blocks nothing (index loads, h2sh
and output writes ride the Activation queue instead). exp() is written as
adjacent duplicate pairs so the fp16 payload multiply runs in DVE 2x
mode.
"""
import sys

sys.path.insert(0, "/opt/trn_rl_repo")

import numpy as np
import ml_dtypes

import concourse.bass as bass
import concourse.mybir as mybir
import concourse.tile as tile
from concourse import bacc
from concourse.bass_utils import run_bass_kernel_spmd

DT2 = mybir.dt.float16
DT8 = mybir.dt.float8e4
F32 = mybir.dt.float32
I16 = mybir.dt.int16
ALU = mybir.AluOpType
ACTF = mybir.ActivationFunctionType

NCORES = 8
P = 128
ASENT = -10000.0
CAPR = 56  # max rounds per gather group (SBUF budget)


class Cfg:
    def __init__(self, n_nodes=50000, in_f=256, hid=64, heads=4, labels=32,
                 lo_limit=32768):
        self.N = n_nodes
        self.IN_F = in_f
        self.HID = hid
        self.HEADS = heads
        self.LABELS = labels
        self.D1 = heads * hid
        self.D2 = heads * labels
        self.LO = lo_limit
        self.own = -(-n_nodes // NCORES)
        # one reserved pad slot (block 0 slot 127) + at least one tail pad
        self.NB = -(-(self.own + 2) // P)
        self.own_pad = self.NB * P
        self.NPOS = NCORES * self.own_pad
        self.NT = self.NPOS // P
        # rows [0, DUP) are duplicated at [NPOS, NPOS+DUP): edges from them
        # may be fetched through either the lo or the hi index window
        self.DUP = max(0, min(self.LO, self.LO + 32768 - self.NPOS,
                              self.NPOS))
        if self.NPOS <= self.LO:
            self.DUP = 0
        self.TROWS = self.NPOS + self.DUP
        # dup region of table2 is written by a second AllGather covering
        # the whole first collective group
        if self.DUP:
            lo_blocks = min(self.LO // (NCORES * P), self.NB)
            self.DUPREG = NCORES * P * min(16, max(lo_blocks, 1))
        else:
            self.DUPREG = 0
        self.TROWS2 = self.NPOS + self.DUPREG
        # table1 fp16 row: [4*(hid) | a_s (H)] ; pitch = 256B multiple
        self.R1USE = self.D1 + heads
        self.ROW1 = -(-(2 * self.R1USE) // 256) * 128  # pitch in fp16 els
        # table2 fp8 row: [4*(labels) | a_s2 as 2 fp8 bytes per head]
        self.R2USE = self.D2 + 2 * heads             # packed bytes
        self.ROW2 = -(-self.R2USE // 256) * 256      # gather pitch (bytes)
        self.W1C = self.D1 + 2 * heads
        self.W2C = self.D2 + 2 * heads
        self.HB1 = 2 + hid
        self.HB2 = 2 + labels
        self.RW1 = heads * self.HB1
        self.RW2 = heads * self.HB2


def _wrap_idx(idx):
    """idx [n] (n%16==0) -> [128, n//16] int16 (16-row wrap, tiled x8)."""
    n = len(idx)
    return np.tile(np.asarray(idx, np.int16).reshape(n // 16, 16).T, (8, 1))


def _ag_sizes(cfg):
    """AllGather group sizes (in blocks). The cumulative row count of the
    groups crosses cfg.LO exactly at a group boundary when possible; the
    final group is a single block so the tail collective is tiny."""
    NB = cfg.NB
    rows_per_blk = NCORES * P
    lo_blocks = min(cfg.LO // rows_per_blk, NB)
    sizes = []

    def chunk(n, maxsz):
        while n > 0:
            t = min(maxsz, n)
            sizes.append(t)
            n -= t

    chunk(lo_blocks, 8)
    rest = NB - lo_blocks
    if rest > 1:
        chunk(rest - 1, 8)
        sizes.append(1)
    elif rest == 1:
        sizes.append(1)
    return sizes


class HostPrep:
    def __init__(self, cfg, edge_index):
        self.cfg = cfg
        N, NB = cfg.N, cfg.NB
        ei = np.asarray(edge_index, np.int64)
        src = np.concatenate([ei[0], np.arange(N, dtype=np.int64)])
        dst = np.concatenate([ei[1], np.arange(N, dtype=np.int64)])
        deg = np.bincount(dst, minlength=N)

        self.ag = np.asarray(_ag_sizes(cfg), np.int64)
        ag_start = np.concatenate([[0], np.cumsum(self.ag)])[:-1]
        self.ag_start = ag_start
        # packed table2 row base of (ag, core): ag_base[a] + c*ag[a]*P
        self.ag_base = np.concatenate(
            [[0], np.cumsum([s * NCORES * P for s in self.ag])])[:-1]
        blk_ag = np.zeros(NB, np.int64)
        for a, s in enumerate(self.ag):
            blk_ag[ag_start[a]:ag_start[a] + s] = a

        # --- phase 1: degree-balanced core deal (snake) -------------------
        order = np.argsort(-deg, kind="stable")
        node_core = np.empty(N, np.int64)
        k = np.arange(N)
        snake = np.where((k // NCORES) % 2 == 0, k % NCORES,
                         NCORES - 1 - (k % NCORES))
        node_core[order] = snake

        def ranks_from_key(key):
            rank = np.empty(N, np.int64)
            for c in range(NCORES):
                nodes = np.where(node_core == c)[0]
                o = np.argsort(key[nodes], kind="stable")
                rr = np.arange(len(nodes))
                # skip reserved pad slot: block0 slot 127
                rr = rr + (rr >= P - 1)
                rank[nodes[o]] = rr
            return rank

        def rows_from_rank(rank):
            csrc = node_core[src]
            cdst = node_core[dst]
            rp1 = ((csrc - cdst) % NCORES) * cfg.own_pad + rank[src]
            blk_s = rank[src] // P
            a_s = blk_ag[blk_s]
            g2r = (self.ag_base[a_s] + csrc * (self.ag[a_s] * P)
                   + (blk_s - ag_start[a_s]) * P + rank[src] % P)
            return rp1, g2r

        def region(r):
            # 0 = flex (duplicated), 1 = lo-fixed, 2 = hi-fixed
            return np.where(r < cfg.DUP, 0, np.where(r < cfg.LO, 1, 2))

        B51 = 64
        rank = ranks_from_key(deg)
        for _ in range(3):
            rp1, g2r = rows_from_rank(rank)
            r1, r2 = region(rp1), region(g2r)
            l1 = np.bincount(dst[r1 == 1], minlength=N).clip(0, B51 - 1)
            h1 = np.bincount(dst[r1 == 2], minlength=N).clip(0, B51 - 1)
            l2 = np.bincount(dst[r2 == 1], minlength=N).clip(0, B51 - 1)
            key = (deg.clip(0, B51 - 1) * B51**3 + l1 * B51**2
                   + h1 * B51 + l2)
            rank = ranks_from_key(-key)
        rp1, g2r = rows_from_rank(rank)

        self.node_core = node_core
        self.rank = rank
        ecore = node_core[dst]
        eblk = rank[dst] // P
        eslot = rank[dst] % P

        # sentinels (block0 slot127 pad; tail pads in last block)
        # L1 (rotated): own b0s127 row=127 (lo); core+k b0s127 first >= LO
        self.sent1 = {0: P - 1, 1: -1}
        for k2 in range(1, NCORES):
            r = k2 * cfg.own_pad + P - 1
            if r >= cfg.LO:
                self.sent1[1] = r - cfg.LO
                break
        # L2 (global packed): core0 b0 s127 (lo); tail pad of last ag (hi)
        self.sent2 = {0: P - 1, 1: -1}
        r = int(self.ag_base[-1] + 0 + (NB - 1 - ag_start[-1]) * P + P - 1)
        if r >= cfg.LO:
            self.sent2[1] = r - cfg.LO
        else:
            self.sent2[1] = self.sent2[0]  # table fits in lo; unused

        # --- rounds structure with flex balancing, shared over cores ------
        def build(rp, lo_assign, sent):
            Rr = np.zeros((NCORES, 2, NB), np.int64)
            okey = (ecore * 2 + (~lo_assign).astype(np.int64)) * NB * P \
                + eblk * P + eslot
            o = np.argsort(okey, kind="stable")
            so = okey[o]
            rv = rp[o]
            runstart = np.r_[True, so[1:] != so[:-1]]
            runid = np.cumsum(runstart) - 1
            first = np.where(runstart)[0]
            rno = np.arange(len(so)) - first[runid]
            kc = so // (2 * NB * P)
            kp = (so // (NB * P)) % 2
            kb = (so // P) % NB
            ks = so % P
            np.maximum.at(Rr, (kc, kp, kb), rno + 1)
            Rsh = Rr.max(axis=0)  # [2, NB] shared across cores
            Rmax = int(Rsh.max()) if Rsh.size else 0
            big = np.full((NCORES, 2, NB, max(Rmax, 1), P), -1, np.int64)
            big[kc, kp, kb, rno, ks] = rv
            idx = {}
            for c in range(NCORES):
                for pss in range(2):
                    for b in range(NB):
                        R = int(Rsh[pss][b])
                        if R:
                            a = big[c, pss, b, :R].copy()
                            a[a < 0] = sent[pss]
                            idx[(c, pss, b)] = a
            return Rsh, idx

        def assign_pass(rp):
            """Per-edge lo/hi assignment: fixed by region, flex balanced
            per (core, block) to minimize Tlo+Thi."""
            reg = region(rp)
            cnt = np.zeros((3, NCORES, NB, P), np.int64)
            np.add.at(cnt, (reg, ecore, eblk, eslot), 1)
            a_of = np.zeros((NCORES, NB, P), np.int64)
            for c in range(NCORES):
                for b in range(NB):
                    f, lmin, h = cnt[0, c, b], cnt[1, c, b], cnt[2, c, b]
                    lo0 = int(lmin.max())
                    hi0 = int(h.max())
                    best = None
                    for Thi in range(hi0, int((h + f).max()) + 1):
                        a_min = np.maximum(h + f - Thi, 0)
                        Tlo = max(lo0, int((lmin + a_min).max()))
                        if best is None or Tlo + Thi < best[0]:
                            best = (Tlo + Thi, np.minimum(a_min, f))
                        if Tlo == lo0:
                            break
                    a_of[c, b] = best[1]
            # per-edge: flex edge j-th of its slot -> lo if j < a_of
            fi = np.where(reg == 0)[0]
            fkey = (ecore[fi] * NB + eblk[fi]) * P + eslot[fi]
            o = np.argsort(fkey, kind="stable")
            so = fkey[o]
            runstart = np.r_[True, so[1:] != so[:-1]] if len(so) \
                else np.zeros(0, bool)
            runid = np.cumsum(runstart) - 1
            first = np.where(runstart)[0]
            jn = np.arange(len(so)) - first[runid] if len(so) \
                else np.zeros(0, np.int64)
            lo_assign = reg == 1
            sel = fi[o]
            lo_assign[sel] = jn < a_of[ecore[sel], eblk[sel], eslot[sel]]
            return lo_assign

        def eff_row(rp, lo_assign):
            """index value: lo window row, or hi window row (dup if flex)."""
            hi_row = np.where(rp < cfg.DUP, rp + cfg.NPOS, rp)
            return np.where(lo_assign, rp, hi_row - cfg.LO)

        la1 = assign_pass(rp1)
        self.R1, idx1 = build(eff_row(rp1, la1), la1, self.sent1)
        la2 = assign_pass(g2r)
        self.R2, idx2 = build(eff_row(g2r, la2), la2, self.sent2)

        # --- gather groups per layer (greedy, within AG boundaries) -------
        def mkgroups(Rsh):
            groups = []
            cur = []
            cur_r = 0
            for b in range(NB):
                rb = int(Rsh[0][b] + Rsh[1][b])
                if cur and (cur_r + rb > CAPR or blk_ag[b] != blk_ag[cur[0]]):
                    groups.append(cur)
                    cur, cur_r = [], 0
                cur.append(b)
                cur_r += rb
            if cur:
                groups.append(cur)
            return groups

        self.groups1 = mkgroups(self.R1)
        self.groups2 = mkgroups(self.R2)

        # --- flattened idx tensors per (core, layer, pass) ----------------
        self.idx_t = {}
        self.gcols = {}
        for lay, (Rsh, idx, groups) in (
                (1, (self.R1, idx1, self.groups1)),
                (2, (self.R2, idx2, self.groups2))):
            for pss in range(2):
                gc = []
                for g in groups:
                    gc.append(int(sum(Rsh[pss][b] for b in g)))
                self.gcols[(lay, pss)] = gc
            for c in range(NCORES):
                for pss in range(2):
                    cols = []
                    for g in groups:
                        for b in g:
                            if Rsh[pss][b]:
                                cols.append(idx[(c, pss, b)].reshape(-1))
                    flat = (np.concatenate(cols) if cols
                            else np.zeros(0, np.int64))
                    if len(flat) == 0:
                        flat = np.full(16, 0, np.int64)
                    self.idx_t[(lay, c, pss)] = _wrap_idx(flat)

        tot_rounds = int(self.R1.sum() + self.R2.sum()) * NCORES
        self.pad_frac = tot_rounds * P / (2 * len(src)) - 1.0


def build_program(cfg, prep, with_bias1, collective=True):
    nc = bacc.Bacc("TRN2", target_bir_lowering=False, debug=False,
                   num_devices=NCORES)
    H = cfg.HEADS
    D1, D2 = cfg.D1, cfg.D2
    NB, NPOS, NT = cfg.NB, cfg.NPOS, cfg.NT
    HID, LB = cfg.HID, cfg.LABELS
    RW1, RW2 = cfg.RW1, cfg.RW2
    K1 = cfg.IN_F // P
    K2 = D1 // P
    W1C, W2C = cfg.W1C, cfg.W2C
    NAG = len(prep.ag)

    n_idx = {}
    for lay in (1, 2):
        for pss in range(2):
            n_idx[(lay, pss)] = prep.idx_t[(lay, 0, pss)].shape[1] * 16

    xT = nc.dram_tensor("xT", [cfg.IN_F, NPOS], DT8, kind="ExternalInput")
    w1e = nc.dram_tensor("w1e", [cfg.IN_F, W1C], DT2, kind="ExternalInput")
    w2e = nc.dram_tensor("w2e", [D1, W2C], DT2, kind="ExternalInput")
    bias1 = nc.dram_tensor("bias1", [1, W1C], F32, kind="ExternalInput")
    bias2 = nc.dram_tensor("bias2", [1, W2C], F32, kind="ExternalInput")
    ones1 = nc.dram_tensor("ones1", [1, P], F32, kind="ExternalInput")
    ident = nc.dram_tensor("ident", [P, P], DT2, kind="ExternalInput")
    # [flag_b0, offs_b0, flag_tail, offs_tail]
    padflag = nc.dram_tensor("padflag", [P, 4], F32, kind="ExternalInput")
    dz = nc.dram_tensor("dz", [P, 8], I16, kind="ExternalInput")
    is_d = {}
    for lay in (1, 2):
        for pss in range(2):
            is_d[(lay, pss)] = nc.dram_tensor(
                f"is{lay}p{pss}", [P, max(n_idx[(lay, pss)] // 16, 16)],
                I16, kind="ExternalInput")
    out = nc.dram_tensor("out", [cfg.own_pad, D2], F32, kind="ExternalOutput")

    with tile.TileContext(nc) as tc:
        with tc.tile_pool(name="dram", bufs=1, space="DRAM") as dram, \
             tc.tile_pool(name="const", bufs=1) as cp:
            table1 = dram.tile([cfg.TROWS, cfg.ROW1], DT2)
            h2sh = [dram.tile([prep.ag[a] * P, cfg.R2USE], DT8,
                              tag=f"h2sh{a}", name=f"h2sh{a}")
                    for a in range(NAG)]
            table2p = dram.tile([NPOS, cfg.R2USE], DT8)
            table2 = dram.tile([cfg.TROWS, cfg.ROW2], DT8)

            def load_const(name, dram_t, shape, dt):
                t = cp.tile(shape, dt, tag=name, name=name + "_sb")
                nc.sync.dma_start(t[:], dram_t[:])
                return t

            # warm-up gather hoists the gpsimd library load to t~0
            dz_sb = load_const("dz", dz, [P, 8], I16)
            warm = cp.tile([P, 1, cfg.ROW1], DT2, tag="warm", name="warm")
            nc.gpsimd.dma_gather(warm[:], table1[:], dz_sb[:], P, P,
                                 cfg.ROW1, single_packet=False)
            ident_sb = load_const("ident", ident, [P, P], DT2)
            bias1_sb = load_const("bias1", bias1, [1, W1C], F32)
            bias2_sb = load_const("bias2", bias2, [1, W2C], F32)
            ones1_sb = load_const("ones1", ones1, [1, P], F32)
            pf_sb = load_const("padflag", padflag, [P, 4], F32)
            w1_sb = [cp.tile([P, W1C], DT2, tag=f"w1_{k}", name=f"w1sb{k}")
                     for k in range(K1)]
            for k in range(K1):
                nc.sync.dma_start(w1_sb[k][:], w1e[k * P:(k + 1) * P, :])
            w2_sb = [cp.tile([P, W2C], DT2, tag=f"w2_{k}", name=f"w2sb{k}")
                     for k in range(K2)]
            for k in range(K2):
                nc.sync.dma_start(w2_sb[k][:], w2e[k * P:(k + 1) * P, :])
            adb1 = cp.tile([P, NB, H], DT2, tag="adb1", name="adb1")
            adb2 = cp.tile([P, NB, H], DT2, tag="adb2", name="adb2")

            def mask_as(out_ap, in_ap, b, eng=None):
                """write a_s, overwriting pad slots with ASENT."""
                eng = eng or nc.vector
                if b == 0:
                    eng.tensor_scalar(out_ap, in_ap, pf_sb[:, 0:1],
                                      pf_sb[:, 1:2], ALU.mult, ALU.add)
                elif b == NB - 1:
                    eng.tensor_scalar(out_ap, in_ap, pf_sb[:, 2:3],
                                      pf_sb[:, 3:4], ALU.mult, ALU.add)
                else:
                    eng.tensor_copy(out_ap, in_ap)

            # ---------------- Phase A: dense layer 1 (replicated) ---------
            SEG = 32
            with tc.tile_pool(name="dA", bufs=3) as dp, \
                 tc.tile_pool(name="dAp", bufs=4, space="PSUM") as dpp:
                RB = 8
                for seg in range(0, NT, SEG):
                    ntile = min(SEG, NT - seg)
                    xs = [dp.tile([P, ntile * P], DT8, tag=f"xs{k}",
                                  name=f"xs{k}") for k in range(K1)]
                    for k in range(K1):
                        nc.sync.dma_start(
                            xs[k][:],
                            xT[k * P:(k + 1) * P, seg * P:(seg + ntile) * P])
                    for t0 in range(0, ntile, RB):
                        nt = min(RB, ntile - t0)
                        rows = dp.tile([P, nt, cfg.R1USE], DT2, tag="rows")
                        for t in range(t0, t0 + nt):
                            rt = seg + t
                            ps = dpp.tile([P, W1C], F32, tag="ps")
                            for k in range(K1):
                                nc.tensor.matmul(
                                    ps[:], xs[k][:, t * P:(t + 1) * P],
                                    w1_sb[k][:], start=(k == 0),
                                    stop=(k == K1 - 1 and not with_bias1))
                            if with_bias1:
                                nc.tensor.matmul(ps[:], ones1_sb[:],
                                                 bias1_sb[:], start=False,
                                                 stop=True)
                            j = t - t0
                            bpos = rt % NB
                            ncp = D1 if bpos in (0, NB - 1) else D1 + H
                            if rt % 5 < 2:
                                nc.vector.tensor_copy(rows[:, j, 0:ncp],
                                                      ps[:, 0:ncp])
                            else:
                                nc.scalar.copy(rows[:, j, 0:ncp],
                                               ps[:, 0:ncp])
                            if ncp == D1:
                                mask_as(rows[:, j, D1:D1 + H],
                                        ps[:, D1:D1 + H], bpos)
                            if rt < NB:
                                nc.vector.tensor_copy(
                                    adb1[:, rt, :], ps[:, D1 + H:D1 + 2 * H])
                        gt = seg + t0
                        nc.scalar.dma_start(
                            table1[gt * P:(gt + nt) * P,
                                   0:cfg.R1USE].rearrange(
                                "(t p) c -> p t c", t=nt), rows[:])
                    r0, r1 = seg * P, (seg + ntile) * P
                    if cfg.DUP and r0 < cfg.DUP:
                        r1 = min(r1, cfg.DUP)
                        nc.scalar.dma_start(
                            table1[NPOS + r0:NPOS + r1, 0:cfg.R1USE],
                            table1[r0:r1, 0:cfg.R1USE])

            # ---------------- Edge phase helper ---------------------------
            def edge_layer(lay, table, DL, RWx, adb, rowlen, postproc,
                           bp, sp, bpp):
                """rowlen in table-dtype elements; payload head-major."""
                tdt = DT2 if lay == 1 else DT8
                Rsh = prep.R1 if lay == 1 else prep.R2
                groups = prep.groups1 if lay == 1 else prep.groups2
                col = {0: 0, 1: 0}
                for gi, g in enumerate(groups):
                    gt = {}
                    for pss in (0, 1):
                        ncols = prep.gcols[(lay, pss)][gi]
                        if ncols == 0:
                            continue
                        n = ncols * P
                        ist = bp.tile([P, n // 16], I16, tag=f"ist{pss}")
                        nc.scalar.dma_start(
                            ist[:], is_d[(lay, pss)][:, col[pss]:col[pss]
                                                     + n // 16])
                        col[pss] += n // 16
                        tbl = (table[0:min(cfg.LO, NPOS), :] if pss == 0
                               else table[cfg.LO:, :])
                        gtile = bp.tile([P, ncols, rowlen], tdt,
                                        tag=f"g{pss}")
                        nc.gpsimd.dma_gather(
                            gtile[:], tbl, ist[:], n, n, rowlen,
                            single_packet=False)
                        gt[pss] = gtile

                    off = {0: 0, 1: 0}
                    for b in g:
                        Rl = int(Rsh[0][b])
                        Rh = int(Rsh[1][b])
                        ps1 = bpp.tile([P, RWx], F32, tag="ps1")
                        first = True
                        for pss, Rn in ((0, Rl), (1, Rh)):
                            if Rn == 0:
                                continue
                            gsl = gt[pss][:, off[pss]:off[pss] + Rn, :]
                            off[pss] += Rn
                            rhs = sp.tile([P, Rn, RWx], DT2, tag="rhs")
                            if lay == 1:
                                asl = gsl[:, :, DL:DL + H]
                            else:
                                asl = gsl[:, :, DL:DL + 2 * H].bitcast(DT2)
                            lg = sp.tile([P, Rn, H], DT2, tag="lg")
                            nc.vector.tensor_tensor(
                                out=lg[:], in0=asl,
                                in1=adb[:, b, None, :].broadcast_to(
                                    [P, Rn, H]),
                                op=ALU.add)
                            lr2 = sp.tile([P, Rn, H], DT2, tag="lr2")
                            nc.vector.tensor_scalar_mul(lr2[:], lg[:], 0.2)
                            nc.vector.tensor_tensor(
                                out=lg[:], in0=lg[:], in1=lr2[:], op=ALU.max)
                            rh4 = rhs[:].rearrange("p r (h c) -> p r h c",
                                                   h=H)
                            nc.scalar.activation(
                                rh4[:, :, :, 0:2],
                                lg[:, :, :, None].broadcast_to(
                                    [P, Rn, H, 2]),
                                ACTF.Exp)
                            g4 = gsl[:, :, 0:DL].rearrange(
                                "p r (h c) -> p r h c", h=H)
                            dph = DL // H
                            nc.vector.tensor_tensor(
                                out=rh4[:, :, :, 2:2 + dph].rearrange(
                                    "p r h (a b) -> p r h a b", b=2),
                                in0=g4[:].rearrange(
                                    "p r h (a b) -> p r h a b", b=2),
                                in1=rh4[:, :, :, 0:2][:, :, :, None, :]
                                .broadcast_to([P, Rn, H, dph // 2, 2]),
                                op=ALU.mult)
                            for r in range(Rn):
                                last = (pss == 1 or Rh == 0) and r == Rn - 1
                                nc.tensor.matmul(
                                    ps1[:], ident_sb[:], rhs[:, r, :],
                                    start=first, stop=last)
                                first = False
                        postproc(b, ps1)

            # ---- Phase B: layer-1 edges + layer-2 dense -------------------
            with tc.tile_pool(name="B", bufs=2) as bp, \
                 tc.tile_pool(name="Bs", bufs=2) as sp, \
                 tc.tile_pool(name="Bp", bufs=3, space="PSUM") as bpp, \
                 tc.tile_pool(name="Bp2", bufs=2, space="PSUM") as bpp2:

                h2acc = {"t": None}

                def post1(b, ps1):
                    p4 = ps1[:].rearrange("p (h c) -> p h c", h=H)
                    dn = sp.tile([P, H], F32, tag="dn")
                    nc.vector.tensor_scalar_add(dn[:], p4[:, :, 0], 1e-16)
                    rc = sp.tile([P, H], F32, tag="rc")
                    nc.vector.reciprocal(rc[:], dn[:])
                    o1 = sp.tile([P, D1], F32, tag="o1")
                    nc.vector.tensor_tensor(
                        out=o1[:].rearrange("p (h d) -> p h d", h=H),
                        in0=p4[:, :, 2:2 + HID],
                        in1=rc[:][:, :, None].broadcast_to([P, H, HID]),
                        op=ALU.mult)
                    # sfull = elu(o1)+1 = min(exp(o1),1) + relu(o1)
                    exf = sp.tile([P, D1], DT2, tag="exf")
                    nc.scalar.activation(exf[:], o1[:], ACTF.Exp)
                    exm = sp.tile([P, D1], DT2, tag="exm")
                    nc.vector.tensor_scalar_min(exm[:], exf[:], 1.0)
                    r1 = sp.tile([P, D1], DT2, tag="r1")
                    nc.scalar.activation(r1[:], o1[:], ACTF.Relu)
                    sfull = sp.tile([P, D1], DT2, tag="sfull")
                    nc.vector.tensor_tensor(
                        out=sfull[:], in0=exm[:], in1=r1[:], op=ALU.add)
                    ps2 = bpp2.tile([P, W2C], F32, tag="ps2")
                    for k in range(K2):
                        pt = bpp2.tile([P, P], DT2, tag="pt")
                        nc.tensor.transpose(
                            pt[:], sfull[:, k * P:(k + 1) * P], ident_sb[:])
                        st = sp.tile([P, P], DT2, tag="st")
                        nc.scalar.copy(st[:], pt[:])
                        nc.tensor.matmul(ps2[:], st[:], w2_sb[k][:],
                                         start=(k == 0), stop=False)
                    nc.tensor.matmul(ps2[:], ones1_sb[:], bias2_sb[:],
                                     start=False, stop=True)
                    # packed h2 row: [h2 fp8 | a_s2 f32->fp16 bitcast]
                    a = int(np.searchsorted(prep.ag_start, b, "right")) - 1
                    pos = b - int(prep.ag_start[a])
                    nbl = prep.ag[a]
                    if pos == 0:
                        h2acc["t"] = bp.tile([P, nbl, cfg.R2USE], DT8,
                                             tag="h2acc", name="h2acc")
                    nc.scalar.copy(h2acc["t"][:, pos, 0:D2], ps2[:, 0:D2])
                    mask_as(h2acc["t"][:, pos, D2:D2 + 2 * H].bitcast(DT2),
                            ps2[:, D2:D2 + H], b)
                    nc.vector.tensor_copy(adb2[:, b, :],
                                          ps2[:, D2 + H:D2 + 2 * H])
                    if pos == nbl - 1:
                        rows_n = nbl * P
                        nc.scalar.dma_start(
                            h2sh[a][0:rows_n, :].rearrange(
                                "(t p) c -> p t c", t=nbl), h2acc["t"][:])
                        gbase = int(prep.ag_base[a])
                        nrow = NCORES * rows_n
                        if collective:
                            nc.gpsimd.collective_compute(
                                "AllGather", ALU.bypass,
                                replica_groups=[list(range(NCORES))],
                                ins=[h2sh[a][0:rows_n, :].opt()],
                                outs=[table2p[gbase:gbase + nrow, :].opt()],
                            )
                        else:
                            for rr in range(NCORES):
                                bs = gbase + rr * rows_n
                                nc.sync.dma_start(
                                    table2p[bs:bs + rows_n, :],
                                    h2sh[a][0:rows_n, :])
                        # expansion runs on the sync queue, which carries
                        # ONLY collective-dependent copies during the edge
                        # phases: its head-of-line wait blocks nothing else
                        nc.sync.dma_start(
                            table2[gbase:gbase + nrow, 0:cfg.R2USE],
                            table2p[gbase:gbase + nrow, :])
                        if cfg.DUP and gbase < cfg.DUP <= gbase + nrow:
                            nc.sync.dma_start(
                                table2[NPOS:NPOS + cfg.DUP, 0:cfg.R2USE],
                                table2p[0:cfg.DUP, :])

                edge_layer(1, table1, D1, RW1, adb1, cfg.ROW1, post1,
                           bp, sp, bpp)

            # --------------- Phase D: layer-2 edges -----------------------
            with tc.tile_pool(name="D", bufs=2) as bp, \
                 tc.tile_pool(name="Ds", bufs=2) as sp, \
                 tc.tile_pool(name="Dp", bufs=3, space="PSUM") as bpp:

                def post2(b, ps1):
                    p4 = ps1[:].rearrange("p (h c) -> p h c", h=H)
                    dn = sp.tile([P, H], F32, tag="dn")
                    nc.vector.tensor_scalar_add(dn[:], p4[:, :, 0], 1e-16)
                    rc = sp.tile([P, H], F32, tag="rc")
                    nc.vector.reciprocal(rc[:], dn[:])
                    o2 = sp.tile([P, D2], F32, tag="o2")
                    nc.vector.tensor_tensor(
                        out=o2[:].rearrange("p (h d) -> p h d", h=H),
                        in0=p4[:, :, 2:2 + LB],
                        in1=rc[:][:, :, None].broadcast_to([P, H, LB]),
                        op=ALU.mult)
                    en = sp.tile([P, D2], F32, tag="en")
                    nc.scalar.activation(en[:], o2[:], ACTF.Exp, scale=-1.0)
                    nc.vector.tensor_scalar_add(en[:], en[:], 1.0)
                    sg = sp.tile([P, D2], F32, tag="sg")
                    nc.vector.reciprocal(sg[:], en[:])
                    nc.scalar.dma_start(out[b * P:(b + 1) * P, :], sg[:])

                edge_layer(2, table2, D2, RW2, adb2, cfg.ROW2, post2,
                           bp, sp, bpp)

    nc.compile()
    return nc


def make_inputs(cfg, prep, x, W1, att_src1, att_dst1, b1, W2, att_src2,
                att_dst2, b2):
    """Per-core in_maps for the SPMD program."""
    H, HID, LB = cfg.HEADS, cfg.HID, cfg.LABELS
    D1, D2 = cfg.D1, cfg.D2
    W1 = np.asarray(W1, np.float32)
    W2 = np.asarray(W2, np.float32)
    as1 = np.asarray(att_src1, np.float32)
    ad1 = np.asarray(att_dst1, np.float32)
    as2 = np.asarray(att_src2, np.float32)
    ad2 = np.asarray(att_dst2, np.float32)
    b1 = np.asarray(b1, np.float32)
    b2 = np.asarray(b2, np.float32)

    # head-major payload: row = [h0 (HID) | h1 | h2 | h3 | a_s]; W columns
    # are already head-major (reshape (H, HID)) so plain concat works.
    A_s1 = np.einsum("ihc,hc->ih", W1.reshape(-1, H, HID), as1)
    A_d1 = np.einsum("ihc,hc->ih", W1.reshape(-1, H, HID), ad1)
    w1e = np.concatenate([W1, A_s1, A_d1], axis=1).astype(np.float16)
    b1h = b1.reshape(H, HID)
    bias1_row = np.concatenate(
        [b1, np.einsum("hc,hc->h", b1h, as1), np.einsum("hc,hc->h", b1h, ad1)]
    ).astype(np.float32)[None, :]

    A_s2 = np.einsum("ihc,hc->ih", W2.reshape(-1, H, LB), as2)
    A_d2 = np.einsum("ihc,hc->ih", W2.reshape(-1, H, LB), ad2)
    w2e_f = np.concatenate([W2, A_s2, A_d2], axis=1)
    b2h = b2.reshape(H, LB)
    bias2_row = (np.concatenate(
        [b2, np.einsum("hc,hc->h", b2h, as2), np.einsum("hc,hc->h", b2h, ad2)])
                 - w2e_f.sum(axis=0)).astype(np.float32)[None, :]
    w2e = w2e_f.astype(np.float16)

    ident = np.eye(P, dtype=np.float16)
    ones1 = np.ones((1, P), np.float32)
    padflag = np.zeros((P, 4), np.float32)
    padflag[:, 0] = 1.0
    padflag[P - 1, 0] = 0.0
    padflag[P - 1, 1] = ASENT
    # tail block: slots [used_tail .. P) unused
    ncount = np.bincount(prep.node_core, minlength=NCORES)
    assert ncount.max() - ncount.min() <= 1
    used_tail = int(ncount.max()) - (P - 1) - (cfg.NB - 2) * P
    padflag[:, 2] = 1.0
    if used_tail < P:
        padflag[used_tail:, 2] = 0.0
        padflag[used_tail:, 3] = ASENT

    # global position-ordered xT (tile-major), then per-core rotation
    x8 = np.asarray(x, np.float32)
    gpos = prep.node_core * cfg.own_pad + prep.rank
    xg = np.zeros((cfg.NPOS, cfg.IN_F), np.float32)
    xg[gpos] = x8
    xTg = np.ascontiguousarray(xg.T).astype(ml_dtypes.float8_e4m3fn)

    in_maps = []
    for c in range(NCORES):
        xTc = np.ascontiguousarray(np.roll(xTg, -c * cfg.own_pad, axis=1))
        m = {
            "xT": xTc,
            "w1e": w1e, "w2e": w2e,
            "bias1": bias1_row, "bias2": bias2_row,
            "ones1": ones1, "ident": ident, "padflag": padflag,
            "dz": np.zeros((P, 8), np.int16),
        }
        for lay in (1, 2):
            for pss in range(2):
                m[f"is{lay}p{pss}"] = prep.idx_t[(lay, c, pss)]
        in_maps.append(m)
    return in_maps, bool(np.any(b1 != 0))


def assemble_output(cfg, prep, results):
    big = np.concatenate([results[c]["out"] for c in range(NCORES)], axis=0)
    gpos = prep.node_core * cfg.own_pad + prep.rank
    return np.ascontiguousarray(big[gpos]).astype(np.float32)


_CACHE = {}


def _get_program(cfg, prep, with_bias1):
    key = (cfg.N, cfg.IN_F, cfg.HEADS, cfg.HID, cfg.LABELS, with_bias1,
           tuple(prep.R1.reshape(-1)), tuple(prep.R2.reshape(-1)))
    if key not in _CACHE:
        _CACHE[key] = build_program(cfg, prep, with_bias1)
    return _CACHE[key]


def kernel(x, edge_index, W1, att_src1, att_dst1, b1, W2, att_src2, att_dst2,
           b2):
    x = np.asarray(x)
    cfg = Cfg(n_nodes=x.shape[0], in_f=x.shape[1],
              hid=np.asarray(att_src1).shape[1],
              heads=np.asarray(att_src1).shape[0],
              labels=np.asarray(att_src2).shape[1])
    prep = HostPrep(cfg, np.asarray(edge_index))
    in_maps, with_bias1 = make_inputs(cfg, prep, x, W1, att_src1, att_dst1,
                                      b1, W2, att_src2, att_dst2, b2)
    nc = _get_program(cfg, prep, with_bias1)
    res = run_bass_kernel_spmd(nc, in_maps, core_ids=list(range(NCORES)))
    return assemble_output(cfg, prep, res.results)


# revision 39
# speedup vs baseline: 1.0115x; 1.0115x over previous
"""GAT (2-layer, 4-head) forward on 8 Trainium2 NeuronCores (Bass/Tile).

v3 design — slot-aligned rounds:
Destination nodes are dealt to 8 cores (degree-balanced snake deal) and,
within each core, lex-sorted by (deg_lo1, deg_hi1, deg_lo2) into blocks of
128 slots. Edge processing is round-based: round r of block b holds, at
partition s, the r-th incoming edge of the node at slot s (sentinel row
pads, whose a_s = -1e4 makes exp() == 0). The baseline's one-hot scatter
matrices disappear: each round accumulates into the block PSUM through an
identity matmul, and a_d lives in SBUF per-block tiles aligned to
partitions (no per-edge a_d gather at all). Rounds split into lo/hi passes
by int16 index reach. Layer-1 table rows are fp16 [4x(64 h) | a_s] 768B,
rotated so own nodes come first; xT ships fp8. Layer-2 rows are fp8
[4x(32 h2) | a_s2 fp16] packed to 136B for a pipelined AllGather in groups
sized [16,16,8,8,1] blocks (the int16 boundary falls exactly between
groups), then locally expanded to 256B-stride gather rows. exp() is
written as adjacent pairs so payload multiplies run in DVE 2x mode.
"""
import sys

sys.path.insert(0, "/opt/trn_rl_repo")

import numpy as np
import ml_dtypes

import concourse.bass as bass
import concourse.mybir as mybir
import concourse.tile as tile
from concourse import bacc
from concourse.bass_utils import run_bass_kernel_spmd

DT2 = mybir.dt.float16
DT8 = mybir.dt.float8e4
F32 = mybir.dt.float32
I16 = mybir.dt.int16
ALU = mybir.AluOpType
ACTF = mybir.ActivationFunctionType

NCORES = 8
P = 128
ASENT = -10000.0
CAPR = 56  # max rounds per gather group (SBUF budget)


class Cfg:
    def __init__(self, n_nodes=50000, in_f=256, hid=64, heads=4, labels=32,
                 lo_limit=32768):
        self.N = n_nodes
        self.IN_F = in_f
        self.HID = hid
        self.HEADS = heads
        self.LABELS = labels
        self.D1 = heads * hid
        self.D2 = heads * labels
        self.LO = lo_limit
        self.own = -(-n_nodes // NCORES)
        # one reserved pad slot (block 0 slot 127) + at least one tail pad
        self.NB = -(-(self.own + 2) // P)
        self.own_pad = self.NB * P
        self.NPOS = NCORES * self.own_pad
        self.NT = self.NPOS // P
        # rows [0, DUP) are duplicated at [NPOS, NPOS+DUP): edges from them
        # may be fetched through either the lo or the hi index window
        self.DUP = max(0, min(self.LO, self.LO + 32768 - self.NPOS,
                              self.NPOS))
        if self.NPOS <= self.LO:
            self.DUP = 0
        self.TROWS = self.NPOS + self.DUP
        # dup region of table2 is written by a second AllGather covering
        # the whole first collective group
        if self.DUP:
            lo_blocks = min(self.LO // (NCORES * P), self.NB)
            self.DUPREG = NCORES * P * min(16, max(lo_blocks, 1))
        else:
            self.DUPREG = 0
        self.TROWS2 = self.NPOS + self.DUPREG
        # table1 fp16 row: [4*(hid) | a_s (H)] ; pitch = 256B multiple
        self.R1USE = self.D1 + heads
        self.ROW1 = -(-(2 * self.R1USE) // 256) * 128  # pitch in fp16 els
        # table2 fp8 row: [4*(labels) | a_s2 as 2 fp8 bytes per head]
        self.R2USE = self.D2 + 2 * heads             # packed bytes
        self.ROW2 = -(-self.R2USE // 256) * 256      # gather pitch (bytes)
        self.W1C = self.D1 + 2 * heads
        self.W2C = self.D2 + 2 * heads
        self.HB1 = 2 + hid
        self.HB2 = 2 + labels
        self.RW1 = heads * self.HB1
        self.RW2 = heads * self.HB2


def _wrap_idx(idx):
    """idx [n] (n%16==0) -> [128, n//16] int16 (16-row wrap, tiled x8)."""
    n = len(idx)
    return np.tile(np.asarray(idx, np.int16).reshape(n // 16, 16).T, (8, 1))


def _ag_sizes(cfg):
    """AllGather group sizes (in blocks). The cumulative row count of the
    groups crosses cfg.LO exactly at a group boundary when possible; the
    final group is a single block so the tail collective is tiny."""
    NB = cfg.NB
    rows_per_blk = NCORES * P
    lo_blocks = min(cfg.LO // rows_per_blk, NB)
    sizes = []

    def chunk(n, maxsz):
        while n > 0:
            t = min(maxsz, n)
            sizes.append(t)
            n -= t

    chunk(lo_blocks, 8)
    rest = NB - lo_blocks
    if rest > 1:
        chunk(rest - 1, 8)
        sizes.append(1)
    elif rest == 1:
        sizes.append(1)
    return sizes


class HostPrep:
    def __init__(self, cfg, edge_index):
        self.cfg = cfg
        N, NB = cfg.N, cfg.NB
        ei = np.asarray(edge_index, np.int64)
        src = np.concatenate([ei[0], np.arange(N, dtype=np.int64)])
        dst = np.concatenate([ei[1], np.arange(N, dtype=np.int64)])
        deg = np.bincount(dst, minlength=N)

        self.ag = np.asarray(_ag_sizes(cfg), np.int64)
        ag_start = np.concatenate([[0], np.cumsum(self.ag)])[:-1]
        self.ag_start = ag_start
        # packed table2 row base of (ag, core): ag_base[a] + c*ag[a]*P
        self.ag_base = np.concatenate(
            [[0], np.cumsum([s * NCORES * P for s in self.ag])])[:-1]
        blk_ag = np.zeros(NB, np.int64)
        for a, s in enumerate(self.ag):
            blk_ag[ag_start[a]:ag_start[a] + s] = a

        # --- phase 1: degree-balanced core deal (snake) -------------------
        order = np.argsort(-deg, kind="stable")
        node_core = np.empty(N, np.int64)
        k = np.arange(N)
        snake = np.where((k // NCORES) % 2 == 0, k % NCORES,
                         NCORES - 1 - (k % NCORES))
        node_core[order] = snake

        def ranks_from_key(key):
            rank = np.empty(N, np.int64)
            for c in range(NCORES):
                nodes = np.where(node_core == c)[0]
                o = np.argsort(key[nodes], kind="stable")
                rr = np.arange(len(nodes))
                # skip reserved pad slot: block0 slot 127
                rr = rr + (rr >= P - 1)
                rank[nodes[o]] = rr
            return rank

        def rows_from_rank(rank):
            csrc = node_core[src]
            cdst = node_core[dst]
            rp1 = ((csrc - cdst) % NCORES) * cfg.own_pad + rank[src]
            blk_s = rank[src] // P
            a_s = blk_ag[blk_s]
            g2r = (self.ag_base[a_s] + csrc * (self.ag[a_s] * P)
                   + (blk_s - ag_start[a_s]) * P + rank[src] % P)
            return rp1, g2r

        def region(r):
            # 0 = flex (duplicated), 1 = lo-fixed, 2 = hi-fixed
            return np.where(r < cfg.DUP, 0, np.where(r < cfg.LO, 1, 2))

        B51 = 64
        rank = ranks_from_key(deg)
        for _ in range(3):
            rp1, g2r = rows_from_rank(rank)
            r1, r2 = region(rp1), region(g2r)
            l1 = np.bincount(dst[r1 == 1], minlength=N).clip(0, B51 - 1)
            h1 = np.bincount(dst[r1 == 2], minlength=N).clip(0, B51 - 1)
            l2 = np.bincount(dst[r2 == 1], minlength=N).clip(0, B51 - 1)
            key = (deg.clip(0, B51 - 1) * B51**3 + l1 * B51**2
                   + h1 * B51 + l2)
            rank = ranks_from_key(-key)
        rp1, g2r = rows_from_rank(rank)

        self.node_core = node_core
        self.rank = rank
        ecore = node_core[dst]
        eblk = rank[dst] // P
        eslot = rank[dst] % P

        # sentinels (block0 slot127 pad; tail pads in last block)
        # L1 (rotated): own b0s127 row=127 (lo); core+k b0s127 first >= LO
        self.sent1 = {0: P - 1, 1: -1}
        for k2 in range(1, NCORES):
            r = k2 * cfg.own_pad + P - 1
            if r >= cfg.LO:
                self.sent1[1] = r - cfg.LO
                break
        # L2 (global packed): core0 b0 s127 (lo); tail pad of last ag (hi)
        self.sent2 = {0: P - 1, 1: -1}
        r = int(self.ag_base[-1] + 0 + (NB - 1 - ag_start[-1]) * P + P - 1)
        if r >= cfg.LO:
            self.sent2[1] = r - cfg.LO
        else:
            self.sent2[1] = self.sent2[0]  # table fits in lo; unused

        # --- rounds structure with flex balancing, shared over cores ------
        def build(rp, lo_assign, sent):
            Rr = np.zeros((NCORES, 2, NB), np.int64)
            okey = (ecore * 2 + (~lo_assign).astype(np.int64)) * NB * P \
                + eblk * P + eslot
            o = np.argsort(okey, kind="stable")
            so = okey[o]
            rv = rp[o]
            runstart = np.r_[True, so[1:] != so[:-1]]
            runid = np.cumsum(runstart) - 1
            first = np.where(runstart)[0]
            rno = np.arange(len(so)) - first[runid]
            kc = so // (2 * NB * P)
            kp = (so // (NB * P)) % 2
            kb = (so // P) % NB
            ks = so % P
            np.maximum.at(Rr, (kc, kp, kb), rno + 1)
            Rsh = Rr.max(axis=0)  # [2, NB] shared across cores
            Rmax = int(Rsh.max()) if Rsh.size else 0
            big = np.full((NCORES, 2, NB, max(Rmax, 1), P), -1, np.int64)
            big[kc, kp, kb, rno, ks] = rv
            idx = {}
            for c in range(NCORES):
                for pss in range(2):
                    for b in range(NB):
                        R = int(Rsh[pss][b])
                        if R:
                            a = big[c, pss, b, :R].copy()
                            a[a < 0] = sent[pss]
                            idx[(c, pss, b)] = a
            return Rsh, idx

        def assign_pass(rp):
            """Per-edge lo/hi assignment: fixed by region, flex balanced
            per (core, block) to minimize Tlo+Thi."""
            reg = region(rp)
            cnt = np.zeros((3, NCORES, NB, P), np.int64)
            np.add.at(cnt, (reg, ecore, eblk, eslot), 1)
            a_of = np.zeros((NCORES, NB, P), np.int64)
            for c in range(NCORES):
                for b in range(NB):
                    f, lmin, h = cnt[0, c, b], cnt[1, c, b], cnt[2, c, b]
                    lo0 = int(lmin.max())
                    hi0 = int(h.max())
                    best = None
                    for Thi in range(hi0, int((h + f).max()) + 1):
                        a_min = np.maximum(h + f - Thi, 0)
                        Tlo = max(lo0, int((lmin + a_min).max()))
                        if best is None or Tlo + Thi < best[0]:
                            best = (Tlo + Thi, np.minimum(a_min, f))
                        if Tlo == lo0:
                            break
                    a_of[c, b] = best[1]
            # per-edge: flex edge j-th of its slot -> lo if j < a_of
            fi = np.where(reg == 0)[0]
            fkey = (ecore[fi] * NB + eblk[fi]) * P + eslot[fi]
            o = np.argsort(fkey, kind="stable")
            so = fkey[o]
            runstart = np.r_[True, so[1:] != so[:-1]] if len(so) \
                else np.zeros(0, bool)
            runid = np.cumsum(runstart) - 1
            first = np.where(runstart)[0]
            jn = np.arange(len(so)) - first[runid] if len(so) \
                else np.zeros(0, np.int64)
            lo_assign = reg == 1
            sel = fi[o]
            lo_assign[sel] = jn < a_of[ecore[sel], eblk[sel], eslot[sel]]
            return lo_assign

        def eff_row(rp, lo_assign):
            """index value: lo window row, or hi window row (dup if flex)."""
            hi_row = np.where(rp < cfg.DUP, rp + cfg.NPOS, rp)
            return np.where(lo_assign, rp, hi_row - cfg.LO)

        la1 = assign_pass(rp1)
        self.R1, idx1 = build(eff_row(rp1, la1), la1, self.sent1)
        la2 = assign_pass(g2r)
        self.R2, idx2 = build(eff_row(g2r, la2), la2, self.sent2)

        # --- gather groups per layer (greedy, within AG boundaries) -------
        def mkgroups(Rsh):
            groups = []
            cur = []
            cur_r = 0
            for b in range(NB):
                rb = int(Rsh[0][b] + Rsh[1][b])
                if cur and (cur_r + rb > CAPR or blk_ag[b] != blk_ag[cur[0]]):
                    groups.append(cur)
                    cur, cur_r = [], 0
                cur.append(b)
                cur_r += rb
            if cur:
                groups.append(cur)
            return groups

        self.groups1 = mkgroups(self.R1)
        self.groups2 = mkgroups(self.R2)

        # --- flattened idx tensors per (core, layer, pass) ----------------
        self.idx_t = {}
        self.gcols = {}
        for lay, (Rsh, idx, groups) in (
                (1, (self.R1, idx1, self.groups1)),
                (2, (self.R2, idx2, self.groups2))):
            for pss in range(2):
                gc = []
                for g in groups:
                    gc.append(int(sum(Rsh[pss][b] for b in g)))
                self.gcols[(lay, pss)] = gc
            for c in range(NCORES):
                for pss in range(2):
                    cols = []
                    for g in groups:
                        for b in g:
                            if Rsh[pss][b]:
                                cols.append(idx[(c, pss, b)].reshape(-1))
                    flat = (np.concatenate(cols) if cols
                            else np.zeros(0, np.int64))
                    if len(flat) == 0:
                        flat = np.full(16, 0, np.int64)
                    self.idx_t[(lay, c, pss)] = _wrap_idx(flat)

        tot_rounds = int(self.R1.sum() + self.R2.sum()) * NCORES
        self.pad_frac = tot_rounds * P / (2 * len(src)) - 1.0


def build_program(cfg, prep, with_bias1, collective=True):
    nc = bacc.Bacc("TRN2", target_bir_lowering=False, debug=False,
                   num_devices=NCORES)
    H = cfg.HEADS
    D1, D2 = cfg.D1, cfg.D2
    NB, NPOS, NT = cfg.NB, cfg.NPOS, cfg.NT
    HID, LB = cfg.HID, cfg.LABELS
    RW1, RW2 = cfg.RW1, cfg.RW2
    K1 = cfg.IN_F // P
    K2 = D1 // P
    W1C, W2C = cfg.W1C, cfg.W2C
    NAG = len(prep.ag)

    n_idx = {}
    for lay in (1, 2):
        for pss in range(2):
            n_idx[(lay, pss)] = prep.idx_t[(lay, 0, pss)].shape[1] * 16

    xT = nc.dram_tensor("xT", [cfg.IN_F, NPOS], DT8, kind="ExternalInput")
    w1e = nc.dram_tensor("w1e", [cfg.IN_F, W1C], DT2, kind="ExternalInput")
    w2e = nc.dram_tensor("w2e", [D1, W2C], DT2, kind="ExternalInput")
    bias1 = nc.dram_tensor("bias1", [1, W1C], F32, kind="ExternalInput")
    bias2 = nc.dram_tensor("bias2", [1, W2C], F32, kind="ExternalInput")
    ones1 = nc.dram_tensor("ones1", [1, P], F32, kind="ExternalInput")
    ident = nc.dram_tensor("ident", [P, P], DT2, kind="ExternalInput")
    # [flag_b0, offs_b0, flag_tail, offs_tail]
    padflag = nc.dram_tensor("padflag", [P, 4], F32, kind="ExternalInput")
    dz = nc.dram_tensor("dz", [P, 8], I16, kind="ExternalInput")
    is_d = {}
    for lay in (1, 2):
        for pss in range(2):
            is_d[(lay, pss)] = nc.dram_tensor(
                f"is{lay}p{pss}", [P, max(n_idx[(lay, pss)] // 16, 16)],
                I16, kind="ExternalInput")
    out = nc.dram_tensor("out", [cfg.own_pad, D2], F32, kind="ExternalOutput")

    with tile.TileContext(nc) as tc:
        with tc.tile_pool(name="dram", bufs=1, space="DRAM") as dram, \
             tc.tile_pool(name="const", bufs=1) as cp:
            table1 = dram.tile([cfg.TROWS, cfg.ROW1], DT2)
            h2sh = [dram.tile([prep.ag[a] * P, cfg.R2USE], DT8,
                              tag=f"h2sh{a}", name=f"h2sh{a}")
                    for a in range(NAG)]
            table2p = dram.tile([NPOS, cfg.R2USE], DT8)
            table2 = dram.tile([cfg.TROWS, cfg.ROW2], DT8)

            def load_const(name, dram_t, shape, dt):
                t = cp.tile(shape, dt, tag=name, name=name + "_sb")
                nc.sync.dma_start(t[:], dram_t[:])
                return t

            # warm-up gather hoists the gpsimd library load to t~0
            dz_sb = load_const("dz", dz, [P, 8], I16)
            warm = cp.tile([P, 1, cfg.ROW1], DT2, tag="warm", name="warm")
            nc.gpsimd.dma_gather(warm[:], table1[:], dz_sb[:], P, P,
                                 cfg.ROW1, single_packet=False)
            ident_sb = load_const("ident", ident, [P, P], DT2)
            bias1_sb = load_const("bias1", bias1, [1, W1C], F32)
            bias2_sb = load_const("bias2", bias2, [1, W2C], F32)
            ones1_sb = load_const("ones1", ones1, [1, P], F32)
            pf_sb = load_const("padflag", padflag, [P, 4], F32)
            w1_sb = [cp.tile([P, W1C], DT2, tag=f"w1_{k}", name=f"w1sb{k}")
                     for k in range(K1)]
            for k in range(K1):
                nc.sync.dma_start(w1_sb[k][:], w1e[k * P:(k + 1) * P, :])
            w2_sb = [cp.tile([P, W2C], DT2, tag=f"w2_{k}", name=f"w2sb{k}")
                     for k in range(K2)]
            for k in range(K2):
                nc.sync.dma_start(w2_sb[k][:], w2e[k * P:(k + 1) * P, :])
            adb1 = cp.tile([P, NB, H], DT2, tag="adb1", name="adb1")
            adb2 = cp.tile([P, NB, H], DT2, tag="adb2", name="adb2")

            def mask_as(out_ap, in_ap, b, eng=None):
                """write a_s, overwriting pad slots with ASENT."""
                eng = eng or nc.vector
                if b == 0:
                    eng.tensor_scalar(out_ap, in_ap, pf_sb[:, 0:1],
                                      pf_sb[:, 1:2], ALU.mult, ALU.add)
                elif b == NB - 1:
                    eng.tensor_scalar(out_ap, in_ap, pf_sb[:, 2:3],
                                      pf_sb[:, 3:4], ALU.mult, ALU.add)
                else:
                    eng.tensor_copy(out_ap, in_ap)

            # ---------------- Phase A: dense layer 1 (replicated) ---------
            SEG = 32
            with tc.tile_pool(name="dA", bufs=3) as dp, \
                 tc.tile_pool(name="dAp", bufs=4, space="PSUM") as dpp:
                RB = 16
                for seg in range(0, NT, SEG):
                    ntile = min(SEG, NT - seg)
                    xs = [dp.tile([P, ntile * P], DT8, tag=f"xs{k}",
                                  name=f"xs{k}") for k in range(K1)]
                    for k in range(K1):
                        nc.sync.dma_start(
                            xs[k][:],
                            xT[k * P:(k + 1) * P, seg * P:(seg + ntile) * P])
                    for t0 in range(0, ntile, RB):
                        nt = min(RB, ntile - t0)
                        rows = dp.tile([P, nt, cfg.R1USE], DT2, tag="rows")
                        for t in range(t0, t0 + nt):
                            rt = seg + t
                            ps = dpp.tile([P, W1C], F32, tag="ps")
                            for k in range(K1):
                                nc.tensor.matmul(
                                    ps[:], xs[k][:, t * P:(t + 1) * P],
                                    w1_sb[k][:], start=(k == 0),
                                    stop=(k == K1 - 1 and not with_bias1))
                            if with_bias1:
                                nc.tensor.matmul(ps[:], ones1_sb[:],
                                                 bias1_sb[:], start=False,
                                                 stop=True)
                            j = t - t0
                            bpos = rt % NB
                            ncp = D1 if bpos in (0, NB - 1) else D1 + H
                            if rt % 5 < 2:
                                nc.vector.tensor_copy(rows[:, j, 0:ncp],
                                                      ps[:, 0:ncp])
                            else:
                                nc.scalar.copy(rows[:, j, 0:ncp],
                                               ps[:, 0:ncp])
                            if ncp == D1:
                                mask_as(rows[:, j, D1:D1 + H],
                                        ps[:, D1:D1 + H], bpos)
                            if rt < NB:
                                nc.vector.tensor_copy(
                                    adb1[:, rt, :], ps[:, D1 + H:D1 + 2 * H])
                        gt = seg + t0
                        nc.scalar.dma_start(
                            table1[gt * P:(gt + nt) * P,
                                   0:cfg.R1USE].rearrange(
                                "(t p) c -> p t c", t=nt), rows[:])
                    r0, r1 = seg * P, (seg + ntile) * P
                    if cfg.DUP and r0 < cfg.DUP:
                        r1 = min(r1, cfg.DUP)
                        nc.scalar.dma_start(
                            table1[NPOS + r0:NPOS + r1, 0:cfg.R1USE],
                            table1[r0:r1, 0:cfg.R1USE])

            # ---------------- Edge phase helper ---------------------------
            def edge_layer(lay, table, DL, RWx, adb, rowlen, postproc,
                           bp, sp, bpp):
                """rowlen in table-dtype elements; payload head-major."""
                tdt = DT2 if lay == 1 else DT8
                Rsh = prep.R1 if lay == 1 else prep.R2
                groups = prep.groups1 if lay == 1 else prep.groups2
                col = {0: 0, 1: 0}
                for gi, g in enumerate(groups):
                    gt = {}
                    for pss in (0, 1):
                        ncols = prep.gcols[(lay, pss)][gi]
                        if ncols == 0:
                            continue
                        n = ncols * P
                        ist = bp.tile([P, n // 16], I16, tag=f"ist{pss}")
                        nc.scalar.dma_start(
                            ist[:], is_d[(lay, pss)][:, col[pss]:col[pss]
                                                     + n // 16])
                        col[pss] += n // 16
                        tbl = (table[0:min(cfg.LO, NPOS), :] if pss == 0
                               else table[cfg.LO:, :])
                        gtile = bp.tile([P, ncols, rowlen], tdt,
                                        tag=f"g{pss}")
                        nc.gpsimd.dma_gather(
                            gtile[:], tbl, ist[:], n, n, rowlen,
                            single_packet=False)
                        gt[pss] = gtile

                    off = {0: 0, 1: 0}
                    for b in g:
                        Rl = int(Rsh[0][b])
                        Rh = int(Rsh[1][b])
                        ps1 = bpp.tile([P, RWx], F32, tag="ps1")
                        first = True
                        for pss, Rn in ((0, Rl), (1, Rh)):
                            if Rn == 0:
                                continue
                            gsl = gt[pss][:, off[pss]:off[pss] + Rn, :]
                            off[pss] += Rn
                            rhs = sp.tile([P, Rn, RWx], DT2, tag="rhs")
                            if lay == 1:
                                asl = gsl[:, :, DL:DL + H]
                            else:
                                asl = gsl[:, :, DL:DL + 2 * H].bitcast(DT2)
                            lg = sp.tile([P, Rn, H], DT2, tag="lg")
                            nc.vector.tensor_tensor(
                                out=lg[:], in0=asl,
                                in1=adb[:, b, None, :].broadcast_to(
                                    [P, Rn, H]),
                                op=ALU.add)
                            lr2 = sp.tile([P, Rn, H], DT2, tag="lr2")
                            nc.vector.tensor_scalar_mul(lr2[:], lg[:], 0.2)
                            nc.vector.tensor_tensor(
                                out=lg[:], in0=lg[:], in1=lr2[:], op=ALU.max)
                            rh4 = rhs[:].rearrange("p r (h c) -> p r h c",
                                                   h=H)
                            nc.scalar.activation(
                                rh4[:, :, :, 0:2],
                                lg[:, :, :, None].broadcast_to(
                                    [P, Rn, H, 2]),
                                ACTF.Exp)
                            g4 = gsl[:, :, 0:DL].rearrange(
                                "p r (h c) -> p r h c", h=H)
                            dph = DL // H
                            nc.vector.tensor_tensor(
                                out=rh4[:, :, :, 2:2 + dph].rearrange(
                                    "p r h (a b) -> p r h a b", b=2),
                                in0=g4[:].rearrange(
                                    "p r h (a b) -> p r h a b", b=2),
                                in1=rh4[:, :, :, 0:2][:, :, :, None, :]
                                .broadcast_to([P, Rn, H, dph // 2, 2]),
                                op=ALU.mult)
                            for r in range(Rn):
                                last = (pss == 1 or Rh == 0) and r == Rn - 1
                                nc.tensor.matmul(
                                    ps1[:], ident_sb[:], rhs[:, r, :],
                                    start=first, stop=last)
                                first = False
                        postproc(b, ps1)

            # ---- Phase B: layer-1 edges + layer-2 dense -------------------
            with tc.tile_pool(name="B", bufs=2) as bp, \
                 tc.tile_pool(name="Bs", bufs=2) as sp, \
                 tc.tile_pool(name="Bp", bufs=3, space="PSUM") as bpp, \
                 tc.tile_pool(name="Bp2", bufs=2, space="PSUM") as bpp2:

                h2acc = {"t": None}

                def post1(b, ps1):
                    p4 = ps1[:].rearrange("p (h c) -> p h c", h=H)
                    dn = sp.tile([P, H], F32, tag="dn")
                    nc.vector.tensor_scalar_add(dn[:], p4[:, :, 0], 1e-16)
                    rc = sp.tile([P, H], F32, tag="rc")
                    nc.vector.reciprocal(rc[:], dn[:])
                    o1 = sp.tile([P, D1], F32, tag="o1")
                    nc.vector.tensor_tensor(
                        out=o1[:].rearrange("p (h d) -> p h d", h=H),
                        in0=p4[:, :, 2:2 + HID],
                        in1=rc[:][:, :, None].broadcast_to([P, H, HID]),
                        op=ALU.mult)
                    # sfull = elu(o1)+1 = min(exp(o1),1) + relu(o1)
                    exf = sp.tile([P, D1], DT2, tag="exf")
                    nc.scalar.activation(exf[:], o1[:], ACTF.Exp)
                    exm = sp.tile([P, D1], DT2, tag="exm")
                    nc.vector.tensor_scalar_min(exm[:], exf[:], 1.0)
                    r1 = sp.tile([P, D1], DT2, tag="r1")
                    nc.scalar.activation(r1[:], o1[:], ACTF.Relu)
                    sfull = sp.tile([P, D1], DT2, tag="sfull")
                    nc.vector.tensor_tensor(
                        out=sfull[:], in0=exm[:], in1=r1[:], op=ALU.add)
                    ps2 = bpp2.tile([P, W2C], F32, tag="ps2")
                    for k in range(K2):
                        pt = bpp2.tile([P, P], DT2, tag="pt")
                        nc.tensor.transpose(
                            pt[:], sfull[:, k * P:(k + 1) * P], ident_sb[:])
                        st = sp.tile([P, P], DT2, tag="st")
                        nc.scalar.copy(st[:], pt[:])
                        nc.tensor.matmul(ps2[:], st[:], w2_sb[k][:],
                                         start=(k == 0), stop=False)
                    nc.tensor.matmul(ps2[:], ones1_sb[:], bias2_sb[:],
                                     start=False, stop=True)
                    # packed h2 row: [h2 fp8 | a_s2 f32->fp16 bitcast]
                    a = int(np.searchsorted(prep.ag_start, b, "right")) - 1
                    pos = b - int(prep.ag_start[a])
                    nbl = prep.ag[a]
                    if pos == 0:
                        h2acc["t"] = bp.tile([P, nbl, cfg.R2USE], DT8,
                                             tag="h2acc", name="h2acc")
                    nc.scalar.copy(h2acc["t"][:, pos, 0:D2], ps2[:, 0:D2])
                    mask_as(h2acc["t"][:, pos, D2:D2 + 2 * H].bitcast(DT2),
                            ps2[:, D2:D2 + H], b)
                    nc.vector.tensor_copy(adb2[:, b, :],
                                          ps2[:, D2 + H:D2 + 2 * H])
                    if pos == nbl - 1:
                        rows_n = nbl * P
                        nc.scalar.dma_start(
                            h2sh[a][0:rows_n, :].rearrange(
                                "(t p) c -> p t c", t=nbl), h2acc["t"][:])
                        gbase = int(prep.ag_base[a])
                        nrow = NCORES * rows_n
                        if collective:
                            nc.gpsimd.collective_compute(
                                "AllGather", ALU.bypass,
                                replica_groups=[list(range(NCORES))],
                                ins=[h2sh[a][0:rows_n, :].opt()],
                                outs=[table2p[gbase:gbase + nrow, :].opt()],
                            )
                        else:
                            for rr in range(NCORES):
                                bs = gbase + rr * rows_n
                                nc.sync.dma_start(
                                    table2p[bs:bs + rows_n, :],
                                    h2sh[a][0:rows_n, :])
                        # expansion runs on the sync queue, which carries
                        # ONLY collective-dependent copies during the edge
                        # phases: its head-of-line wait blocks nothing else
                        nc.sync.dma_start(
                            table2[gbase:gbase + nrow, 0:cfg.R2USE],
                            table2p[gbase:gbase + nrow, :])
                        if cfg.DUP and gbase < cfg.DUP <= gbase + nrow:
                            nc.sync.dma_start(
                                table2[NPOS:NPOS + cfg.DUP, 0:cfg.R2USE],
                                table2p[0:cfg.DUP, :])

                edge_layer(1, table1, D1, RW1, adb1, cfg.ROW1, post1,
                           bp, sp, bpp)

            # --------------- Phase D: layer-2 edges -----------------------
            with tc.tile_pool(name="D", bufs=2) as bp, \
                 tc.tile_pool(name="Ds", bufs=2) as sp, \
                 tc.tile_pool(name="Dp", bufs=3, space="PSUM") as bpp:

                def post2(b, ps1):
                    p4 = ps1[:].rearrange("p (h c) -> p h c", h=H)
                    dn = sp.tile([P, H], F32, tag="dn")
                    nc.vector.tensor_scalar_add(dn[:], p4[:, :, 0], 1e-16)
                    rc = sp.tile([P, H], F32, tag="rc")
                    nc.vector.reciprocal(rc[:], dn[:])
                    o2 = sp.tile([P, D2], F32, tag="o2")
                    nc.vector.tensor_tensor(
                        out=o2[:].rearrange("p (h d) -> p h d", h=H),
                        in0=p4[:, :, 2:2 + LB],
                        in1=rc[:][:, :, None].broadcast_to([P, H, LB]),
                        op=ALU.mult)
                    en = sp.tile([P, D2], F32, tag="en")
                    nc.scalar.activation(en[:], o2[:], ACTF.Exp, scale=-1.0)
                    nc.vector.tensor_scalar_add(en[:], en[:], 1.0)
                    sg = sp.tile([P, D2], F32, tag="sg")
                    nc.vector.reciprocal(sg[:], en[:])
                    nc.scalar.dma_start(out[b * P:(b + 1) * P, :], sg[:])

                edge_layer(2, table2, D2, RW2, adb2, cfg.ROW2, post2,
                           bp, sp, bpp)

    nc.compile()
    return nc


def make_inputs(cfg, prep, x, W1, att_src1, att_dst1, b1, W2, att_src2,
                att_dst2, b2):
    """Per-core in_maps for the SPMD program."""
    H, HID, LB = cfg.HEADS, cfg.HID, cfg.LABELS
    D1, D2 = cfg.D1, cfg.D2
    W1 = np.asarray(W1, np.float32)
    W2 = np.asarray(W2, np.float32)
    as1 = np.asarray(att_src1, np.float32)
    ad1 = np.asarray(att_dst1, np.float32)
    as2 = np.asarray(att_src2, np.float32)
    ad2 = np.asarray(att_dst2, np.float32)
    b1 = np.asarray(b1, np.float32)
    b2 = np.asarray(b2, np.float32)

    # head-major payload: row = [h0 (HID) | h1 | h2 | h3 | a_s]; W columns
    # are already head-major (reshape (H, HID)) so plain concat works.
    A_s1 = np.einsum("ihc,hc->ih", W1.reshape(-1, H, HID), as1)
    A_d1 = np.einsum("ihc,hc->ih", W1.reshape(-1, H, HID), ad1)
    w1e = np.concatenate([W1, A_s1, A_d1], axis=1).astype(np.float16)
    b1h = b1.reshape(H, HID)
    bias1_row = np.concatenate(
        [b1, np.einsum("hc,hc->h", b1h, as1), np.einsum("hc,hc->h", b1h, ad1)]
    ).astype(np.float32)[None, :]

    A_s2 = np.einsum("ihc,hc->ih", W2.reshape(-1, H, LB), as2)
    A_d2 = np.einsum("ihc,hc->ih", W2.reshape(-1, H, LB), ad2)
    w2e_f = np.concatenate([W2, A_s2, A_d2], axis=1)
    b2h = b2.reshape(H, LB)
    bias2_row = (np.concatenate(
        [b2, np.einsum("hc,hc->h", b2h, as2), np.einsum("hc,hc->h", b2h, ad2)])
                 - w2e_f.sum(axis=0)).astype(np.float32)[None, :]
    w2e = w2e_f.astype(np.float16)

    ident = np.eye(P, dtype=np.float16)
    ones1 = np.ones((1, P), np.float32)
    padflag = np.zeros((P, 4), np.float32)
    padflag[:, 0] = 1.0
    padflag[P - 1, 0] = 0.0
    padflag[P - 1, 1] = ASENT
    # tail block: slots [used_tail .. P) unused
    ncount = np.bincount(prep.node_core, minlength=NCORES)
    assert ncount.max() - ncount.min() <= 1
    used_tail = int(ncount.max()) - (P - 1) - (cfg.NB - 2) * P
    padflag[:, 2] = 1.0
    if used_tail < P:
        padflag[used_tail:, 2] = 0.0
        padflag[used_tail:, 3] = ASENT

    # global position-ordered xT (tile-major), then per-core rotation
    x8 = np.asarray(x, np.float32)
    gpos = prep.node_core * cfg.own_pad + prep.rank
    xg = np.zeros((cfg.NPOS, cfg.IN_F), np.float32)
    xg[gpos] = x8
    xTg = np.ascontiguousarray(xg.T).astype(ml_dtypes.float8_e4m3fn)

    in_maps = []
    for c in range(NCORES):
        xTc = np.ascontiguousarray(np.roll(xTg, -c * cfg.own_pad, axis=1))
        m = {
            "xT": xTc,
            "w1e": w1e, "w2e": w2e,
            "bias1": bias1_row, "bias2": bias2_row,
            "ones1": ones1, "ident": ident, "padflag": padflag,
            "dz": np.zeros((P, 8), np.int16),
        }
        for lay in (1, 2):
            for pss in range(2):
                m[f"is{lay}p{pss}"] = prep.idx_t[(lay, c, pss)]
        in_maps.append(m)
    return in_maps, bool(np.any(b1 != 0))


def assemble_output(cfg, prep, results):
    big = np.concatenate([results[c]["out"] for c in range(NCORES)], axis=0)
    gpos = prep.node_core * cfg.own_pad + prep.rank
    return np.ascontiguousarray(big[gpos]).astype(np.float32)


_CACHE = {}


def _get_program(cfg, prep, with_bias1):
    key = (cfg.N, cfg.IN_F, cfg.HEADS, cfg.HID, cfg.LABELS, with_bias1,
           tuple(prep.R1.reshape(-1)), tuple(prep.R2.reshape(-1)))
    if key not in _CACHE:
        _CACHE[key] = build_program(cfg, prep, with_bias1)
    return _CACHE[key]


def kernel(x, edge_index, W1, att_src1, att_dst1, b1, W2, att_src2, att_dst2,
           b2):
    x = np.asarray(x)
    cfg = Cfg(n_nodes=x.shape[0], in_f=x.shape[1],
              hid=np.asarray(att_src1).shape[1],
              heads=np.asarray(att_src1).shape[0],
              labels=np.asarray(att_src2).shape[1])
    prep = HostPrep(cfg, np.asarray(edge_index))
    in_maps, with_bias1 = make_inputs(cfg, prep, x, W1, att_src1, att_dst1,
                                      b1, W2, att_src2, att_dst2, b2)
    nc = _get_program(cfg, prep, with_bias1)
    res = run_bass_kernel_spmd(nc, in_maps, core_ids=list(range(NCORES)))
    return assemble_output(cfg, prep, res.results)


# revision 41
# speedup vs baseline: 1.0124x; 1.0008x over previous
"""GAT (2-layer, 4-head) forward on 8 Trainium2 NeuronCores (Bass/Tile).

v3 design — slot-aligned rounds:
Destination nodes are dealt to 8 cores (degree-balanced snake deal) and,
within each core, lex-sorted by (deg_lo1, deg_hi1, deg_lo2) into blocks of
128 slots. Edge processing is round-based: round r of block b holds, at
partition s, the r-th incoming edge of the node at slot s (sentinel row
pads, whose a_s = -1e4 makes exp() == 0). The baseline's one-hot scatter
matrices disappear: each round accumulates into the block PSUM through an
identity matmul, and a_d lives in SBUF per-block tiles aligned to
partitions (no per-edge a_d gather at all). Rounds split into lo/hi passes
by int16 index reach. Layer-1 table rows are fp16 [4x(64 h) | a_s] 768B,
rotated so own nodes come first; xT ships fp8. Layer-2 rows are fp8
[4x(32 h2) | a_s2 fp16] packed to 136B for a pipelined AllGather in groups
sized [16,16,8,8,1] blocks (the int16 boundary falls exactly between
groups), then locally expanded to 256B-stride gather rows. exp() is
written as adjacent pairs so payload multiplies run in DVE 2x mode.
"""
import sys

sys.path.insert(0, "/opt/trn_rl_repo")

import numpy as np
import ml_dtypes

import concourse.bass as bass
import concourse.mybir as mybir
import concourse.tile as tile
from concourse import bacc
from concourse.bass_utils import run_bass_kernel_spmd

DT2 = mybir.dt.float16
DT8 = mybir.dt.float8e4
F32 = mybir.dt.float32
I16 = mybir.dt.int16
ALU = mybir.AluOpType
ACTF = mybir.ActivationFunctionType

NCORES = 8
P = 128
ASENT = -10000.0
CAPR = 56  # max rounds per gather group (SBUF budget)


class Cfg:
    def __init__(self, n_nodes=50000, in_f=256, hid=64, heads=4, labels=32,
                 lo_limit=32768):
        self.N = n_nodes
        self.IN_F = in_f
        self.HID = hid
        self.HEADS = heads
        self.LABELS = labels
        self.D1 = heads * hid
        self.D2 = heads * labels
        self.LO = lo_limit
        self.own = -(-n_nodes // NCORES)
        # one reserved pad slot (block 0 slot 127) + at least one tail pad
        self.NB = -(-(self.own + 2) // P)
        self.own_pad = self.NB * P
        self.NPOS = NCORES * self.own_pad
        self.NT = self.NPOS // P
        # rows [0, DUP) are duplicated at [NPOS, NPOS+DUP): edges from them
        # may be fetched through either the lo or the hi index window
        self.DUP = max(0, min(self.LO, self.LO + 32768 - self.NPOS,
                              self.NPOS))
        if self.NPOS <= self.LO:
            self.DUP = 0
        self.TROWS = self.NPOS + self.DUP
        # dup region of table2 is written by a second AllGather covering
        # the whole first collective group
        if self.DUP:
            lo_blocks = min(self.LO // (NCORES * P), self.NB)
            self.DUPREG = NCORES * P * min(16, max(lo_blocks, 1))
        else:
            self.DUPREG = 0
        self.TROWS2 = self.NPOS + self.DUPREG
        # table1 fp16 row: [4*(hid) | a_s (H)] ; pitch = 256B multiple
        self.R1USE = self.D1 + heads
        self.ROW1 = -(-(2 * self.R1USE) // 256) * 128  # pitch in fp16 els
        # table2 fp8 row: [4*(labels) | a_s2 as 2 fp8 bytes per head]
        self.R2USE = self.D2 + 2 * heads             # packed bytes
        self.ROW2 = -(-self.R2USE // 256) * 256      # gather pitch (bytes)
        self.W1C = self.D1 + 2 * heads
        self.W2C = self.D2 + 2 * heads
        self.HB1 = 2 + hid
        self.HB2 = 2 + labels
        self.RW1 = heads * self.HB1
        self.RW2 = heads * self.HB2


def _wrap_idx(idx):
    """idx [n] (n%16==0) -> [128, n//16] int16 (16-row wrap, tiled x8)."""
    n = len(idx)
    return np.tile(np.asarray(idx, np.int16).reshape(n // 16, 16).T, (8, 1))


def _ag_sizes(cfg):
    """AllGather group sizes (in blocks). The cumulative row count of the
    groups crosses cfg.LO exactly at a group boundary when possible; the
    final group is a single block so the tail collective is tiny."""
    NB = cfg.NB
    rows_per_blk = NCORES * P
    lo_blocks = min(cfg.LO // rows_per_blk, NB)
    sizes = []

    def chunk(n, maxsz):
        while n > 0:
            t = min(maxsz, n)
            sizes.append(t)
            n -= t

    chunk(lo_blocks, 8)
    rest = NB - lo_blocks
    if rest > 1:
        chunk(rest - 1, 8)
        sizes.append(1)
    elif rest == 1:
        sizes.append(1)
    return sizes


class HostPrep:
    def __init__(self, cfg, edge_index):
        self.cfg = cfg
        N, NB = cfg.N, cfg.NB
        ei = np.asarray(edge_index, np.int64)
        src = np.concatenate([ei[0], np.arange(N, dtype=np.int64)])
        dst = np.concatenate([ei[1], np.arange(N, dtype=np.int64)])
        deg = np.bincount(dst, minlength=N)

        self.ag = np.asarray(_ag_sizes(cfg), np.int64)
        ag_start = np.concatenate([[0], np.cumsum(self.ag)])[:-1]
        self.ag_start = ag_start
        # packed table2 row base of (ag, core): ag_base[a] + c*ag[a]*P
        self.ag_base = np.concatenate(
            [[0], np.cumsum([s * NCORES * P for s in self.ag])])[:-1]
        blk_ag = np.zeros(NB, np.int64)
        for a, s in enumerate(self.ag):
            blk_ag[ag_start[a]:ag_start[a] + s] = a

        # --- phase 1: degree-balanced core deal (snake) -------------------
        order = np.argsort(-deg, kind="stable")
        node_core = np.empty(N, np.int64)
        k = np.arange(N)
        snake = np.where((k // NCORES) % 2 == 0, k % NCORES,
                         NCORES - 1 - (k % NCORES))
        node_core[order] = snake

        def ranks_from_key(key):
            rank = np.empty(N, np.int64)
            for c in range(NCORES):
                nodes = np.where(node_core == c)[0]
                o = np.argsort(key[nodes], kind="stable")
                rr = np.arange(len(nodes))
                # skip reserved pad slot: block0 slot 127
                rr = rr + (rr >= P - 1)
                rank[nodes[o]] = rr
            return rank

        def rows_from_rank(rank):
            csrc = node_core[src]
            cdst = node_core[dst]
            rp1 = ((csrc - cdst) % NCORES) * cfg.own_pad + rank[src]
            blk_s = rank[src] // P
            a_s = blk_ag[blk_s]
            g2r = (self.ag_base[a_s] + csrc * (self.ag[a_s] * P)
                   + (blk_s - ag_start[a_s]) * P + rank[src] % P)
            return rp1, g2r

        def region(r):
            # 0 = flex (duplicated), 1 = lo-fixed, 2 = hi-fixed
            return np.where(r < cfg.DUP, 0, np.where(r < cfg.LO, 1, 2))

        B51 = 64
        rank = ranks_from_key(deg)
        for _ in range(3):
            rp1, g2r = rows_from_rank(rank)
            r1, r2 = region(rp1), region(g2r)
            l1 = np.bincount(dst[r1 == 1], minlength=N).clip(0, B51 - 1)
            h1 = np.bincount(dst[r1 == 2], minlength=N).clip(0, B51 - 1)
            l2 = np.bincount(dst[r2 == 1], minlength=N).clip(0, B51 - 1)
            key = (deg.clip(0, B51 - 1) * B51**3 + l1 * B51**2
                   + h1 * B51 + l2)
            rank = ranks_from_key(-key)
        rp1, g2r = rows_from_rank(rank)

        self.node_core = node_core
        self.rank = rank
        ecore = node_core[dst]
        eblk = rank[dst] // P
        eslot = rank[dst] % P

        # sentinels (block0 slot127 pad; tail pads in last block)
        # L1 (rotated): own b0s127 row=127 (lo); core+k b0s127 first >= LO
        self.sent1 = {0: P - 1, 1: -1}
        for k2 in range(1, NCORES):
            r = k2 * cfg.own_pad + P - 1
            if r >= cfg.LO:
                self.sent1[1] = r - cfg.LO
                break
        # L2 (global packed): core0 b0 s127 (lo); tail pad of last ag (hi)
        self.sent2 = {0: P - 1, 1: -1}
        r = int(self.ag_base[-1] + 0 + (NB - 1 - ag_start[-1]) * P + P - 1)
        if r >= cfg.LO:
            self.sent2[1] = r - cfg.LO
        else:
            self.sent2[1] = self.sent2[0]  # table fits in lo; unused

        # --- rounds structure with flex balancing, shared over cores ------
        def build(rp, lo_assign, sent):
            Rr = np.zeros((NCORES, 2, NB), np.int64)
            okey = (ecore * 2 + (~lo_assign).astype(np.int64)) * NB * P \
                + eblk * P + eslot
            o = np.argsort(okey, kind="stable")
            so = okey[o]
            rv = rp[o]
            runstart = np.r_[True, so[1:] != so[:-1]]
            runid = np.cumsum(runstart) - 1
            first = np.where(runstart)[0]
            rno = np.arange(len(so)) - first[runid]
            kc = so // (2 * NB * P)
            kp = (so // (NB * P)) % 2
            kb = (so // P) % NB
            ks = so % P
            np.maximum.at(Rr, (kc, kp, kb), rno + 1)
            Rsh = Rr.max(axis=0)  # [2, NB] shared across cores
            Rmax = int(Rsh.max()) if Rsh.size else 0
            big = np.full((NCORES, 2, NB, max(Rmax, 1), P), -1, np.int64)
            big[kc, kp, kb, rno, ks] = rv
            idx = {}
            for c in range(NCORES):
                for pss in range(2):
                    for b in range(NB):
                        R = int(Rsh[pss][b])
                        if R:
                            a = big[c, pss, b, :R].copy()
                            a[a < 0] = sent[pss]
                            idx[(c, pss, b)] = a
            return Rsh, idx

        def assign_pass(rp):
            """Per-edge lo/hi assignment: fixed by region, flex balanced
            per (core, block) to minimize Tlo+Thi."""
            reg = region(rp)
            cnt = np.zeros((3, NCORES, NB, P), np.int64)
            np.add.at(cnt, (reg, ecore, eblk, eslot), 1)
            a_of = np.zeros((NCORES, NB, P), np.int64)
            for c in range(NCORES):
                for b in range(NB):
                    f, lmin, h = cnt[0, c, b], cnt[1, c, b], cnt[2, c, b]
                    lo0 = int(lmin.max())
                    hi0 = int(h.max())
                    best = None
                    for Thi in range(hi0, int((h + f).max()) + 1):
                        a_min = np.maximum(h + f - Thi, 0)
                        Tlo = max(lo0, int((lmin + a_min).max()))
                        if best is None or Tlo + Thi < best[0]:
                            best = (Tlo + Thi, np.minimum(a_min, f))
                        if Tlo == lo0:
                            break
                    a_of[c, b] = best[1]
            # per-edge: flex edge j-th of its slot -> lo if j < a_of
            fi = np.where(reg == 0)[0]
            fkey = (ecore[fi] * NB + eblk[fi]) * P + eslot[fi]
            o = np.argsort(fkey, kind="stable")
            so = fkey[o]
            runstart = np.r_[True, so[1:] != so[:-1]] if len(so) \
                else np.zeros(0, bool)
            runid = np.cumsum(runstart) - 1
            first = np.where(runstart)[0]
            jn = np.arange(len(so)) - first[runid] if len(so) \
                else np.zeros(0, np.int64)
            lo_assign = reg == 1
            sel = fi[o]
            lo_assign[sel] = jn < a_of[ecore[sel], eblk[sel], eslot[sel]]
            return lo_assign

        def eff_row(rp, lo_assign):
            """index value: lo window row, or hi window row (dup if flex)."""
            hi_row = np.where(rp < cfg.DUP, rp + cfg.NPOS, rp)
            return np.where(lo_assign, rp, hi_row - cfg.LO)

        la1 = assign_pass(rp1)
        self.R1, idx1 = build(eff_row(rp1, la1), la1, self.sent1)
        la2 = assign_pass(g2r)
        self.R2, idx2 = build(eff_row(g2r, la2), la2, self.sent2)

        # --- gather groups per layer (greedy, within AG boundaries) -------
        def mkgroups(Rsh):
            groups = []
            cur = []
            cur_r = 0
            for b in range(NB):
                rb = int(Rsh[0][b] + Rsh[1][b])
                if cur and (cur_r + rb > CAPR or blk_ag[b] != blk_ag[cur[0]]):
                    groups.append(cur)
                    cur, cur_r = [], 0
                cur.append(b)
                cur_r += rb
            if cur:
                groups.append(cur)
            return groups

        self.groups1 = mkgroups(self.R1)
        self.groups2 = mkgroups(self.R2)

        # --- flattened idx tensors per (core, layer, pass) ----------------
        self.idx_t = {}
        self.gcols = {}
        for lay, (Rsh, idx, groups) in (
                (1, (self.R1, idx1, self.groups1)),
                (2, (self.R2, idx2, self.groups2))):
            for pss in range(2):
                gc = []
                for g in groups:
                    gc.append(int(sum(Rsh[pss][b] for b in g)))
                self.gcols[(lay, pss)] = gc
            for c in range(NCORES):
                for pss in range(2):
                    cols = []
                    for g in groups:
                        for b in g:
                            if Rsh[pss][b]:
                                cols.append(idx[(c, pss, b)].reshape(-1))
                    flat = (np.concatenate(cols) if cols
                            else np.zeros(0, np.int64))
                    if len(flat) == 0:
                        flat = np.full(16, 0, np.int64)
                    self.idx_t[(lay, c, pss)] = _wrap_idx(flat)

        tot_rounds = int(self.R1.sum() + self.R2.sum()) * NCORES
        self.pad_frac = tot_rounds * P / (2 * len(src)) - 1.0


def build_program(cfg, prep, with_bias1, collective=True):
    nc = bacc.Bacc("TRN2", target_bir_lowering=False, debug=False,
                   num_devices=NCORES)
    H = cfg.HEADS
    D1, D2 = cfg.D1, cfg.D2
    NB, NPOS, NT = cfg.NB, cfg.NPOS, cfg.NT
    HID, LB = cfg.HID, cfg.LABELS
    RW1, RW2 = cfg.RW1, cfg.RW2
    K1 = cfg.IN_F // P
    K2 = D1 // P
    W1C, W2C = cfg.W1C, cfg.W2C
    NAG = len(prep.ag)

    n_idx = {}
    for lay in (1, 2):
        for pss in range(2):
            n_idx[(lay, pss)] = prep.idx_t[(lay, 0, pss)].shape[1] * 16

    xT = nc.dram_tensor("xT", [cfg.IN_F, NPOS], DT8, kind="ExternalInput")
    w1e = nc.dram_tensor("w1e", [cfg.IN_F, W1C], DT2, kind="ExternalInput")
    w2e = nc.dram_tensor("w2e", [D1, W2C], DT2, kind="ExternalInput")
    bias1 = nc.dram_tensor("bias1", [1, W1C], F32, kind="ExternalInput")
    bias2 = nc.dram_tensor("bias2", [1, W2C], F32, kind="ExternalInput")
    ones1 = nc.dram_tensor("ones1", [1, P], F32, kind="ExternalInput")
    ident = nc.dram_tensor("ident", [P, P], DT2, kind="ExternalInput")
    # [flag_b0, offs_b0, flag_tail, offs_tail]
    padflag = nc.dram_tensor("padflag", [P, 4], F32, kind="ExternalInput")
    dz = nc.dram_tensor("dz", [P, 8], I16, kind="ExternalInput")
    is_d = {}
    for lay in (1, 2):
        for pss in range(2):
            is_d[(lay, pss)] = nc.dram_tensor(
                f"is{lay}p{pss}", [P, max(n_idx[(lay, pss)] // 16, 16)],
                I16, kind="ExternalInput")
    out = nc.dram_tensor("out", [cfg.own_pad, D2], F32, kind="ExternalOutput")

    with tile.TileContext(nc) as tc:
        with tc.tile_pool(name="dram", bufs=1, space="DRAM") as dram, \
             tc.tile_pool(name="const", bufs=1) as cp:
            table1 = dram.tile([cfg.TROWS, cfg.ROW1], DT2)
            h2sh = [dram.tile([prep.ag[a] * P, cfg.R2USE], DT8,
                              tag=f"h2sh{a}", name=f"h2sh{a}")
                    for a in range(NAG)]
            table2p = dram.tile([NPOS, cfg.R2USE], DT8)
            table2 = dram.tile([cfg.TROWS, cfg.ROW2], DT8)

            def load_const(name, dram_t, shape, dt):
                t = cp.tile(shape, dt, tag=name, name=name + "_sb")
                nc.sync.dma_start(t[:], dram_t[:])
                return t

            # warm-up gather hoists the gpsimd library load to t~0
            dz_sb = load_const("dz", dz, [P, 8], I16)
            warm = cp.tile([P, 1, cfg.ROW1], DT2, tag="warm", name="warm")
            nc.gpsimd.dma_gather(warm[:], table1[:], dz_sb[:], P, P,
                                 cfg.ROW1, single_packet=False)
            ident_sb = load_const("ident", ident, [P, P], DT2)
            bias1_sb = load_const("bias1", bias1, [1, W1C], F32)
            bias2_sb = load_const("bias2", bias2, [1, W2C], F32)
            ones1_sb = load_const("ones1", ones1, [1, P], F32)
            pf_sb = load_const("padflag", padflag, [P, 4], F32)
            w1_sb = [cp.tile([P, W1C], DT2, tag=f"w1_{k}", name=f"w1sb{k}")
                     for k in range(K1)]
            for k in range(K1):
                nc.sync.dma_start(w1_sb[k][:], w1e[k * P:(k + 1) * P, :])
            w2_sb = [cp.tile([P, W2C], DT2, tag=f"w2_{k}", name=f"w2sb{k}")
                     for k in range(K2)]
            for k in range(K2):
                nc.sync.dma_start(w2_sb[k][:], w2e[k * P:(k + 1) * P, :])
            adb1 = cp.tile([P, NB, H], DT2, tag="adb1", name="adb1")
            adb2 = cp.tile([P, NB, H], DT2, tag="adb2", name="adb2")

            def mask_as(out_ap, in_ap, b, eng=None):
                """write a_s, overwriting pad slots with ASENT."""
                eng = eng or nc.vector
                if b == 0:
                    eng.tensor_scalar(out_ap, in_ap, pf_sb[:, 0:1],
                                      pf_sb[:, 1:2], ALU.mult, ALU.add)
                elif b == NB - 1:
                    eng.tensor_scalar(out_ap, in_ap, pf_sb[:, 2:3],
                                      pf_sb[:, 3:4], ALU.mult, ALU.add)
                else:
                    eng.tensor_copy(out_ap, in_ap)

            # ---------------- Phase A: dense layer 1 (replicated) ---------
            SEG = 48
            with tc.tile_pool(name="dA", bufs=3) as dp, \
                 tc.tile_pool(name="dAp", bufs=6, space="PSUM") as dpp:
                RB = 16
                for seg in range(0, NT, SEG):
                    ntile = min(SEG, NT - seg)
                    xs = [dp.tile([P, ntile * P], DT8, tag=f"xs{k}",
                                  name=f"xs{k}") for k in range(K1)]
                    for k in range(K1):
                        nc.sync.dma_start(
                            xs[k][:],
                            xT[k * P:(k + 1) * P, seg * P:(seg + ntile) * P])
                    for t0 in range(0, ntile, RB):
                        nt = min(RB, ntile - t0)
                        rows = dp.tile([P, nt, cfg.R1USE], DT2, tag="rows")
                        for t in range(t0, t0 + nt):
                            rt = seg + t
                            ps = dpp.tile([P, W1C], F32, tag="ps")
                            for k in range(K1):
                                nc.tensor.matmul(
                                    ps[:], xs[k][:, t * P:(t + 1) * P],
                                    w1_sb[k][:], start=(k == 0),
                                    stop=(k == K1 - 1 and not with_bias1))
                            if with_bias1:
                                nc.tensor.matmul(ps[:], ones1_sb[:],
                                                 bias1_sb[:], start=False,
                                                 stop=True)
                            j = t - t0
                            bpos = rt % NB
                            ncp = D1 if bpos in (0, NB - 1) else D1 + H
                            if rt % 5 < 2:
                                nc.vector.tensor_copy(rows[:, j, 0:ncp],
                                                      ps[:, 0:ncp])
                            else:
                                nc.scalar.copy(rows[:, j, 0:ncp],
                                               ps[:, 0:ncp])
                            if ncp == D1:
                                mask_as(rows[:, j, D1:D1 + H],
                                        ps[:, D1:D1 + H], bpos)
                            if rt < NB:
                                nc.vector.tensor_copy(
                                    adb1[:, rt, :], ps[:, D1 + H:D1 + 2 * H])
                        gt = seg + t0
                        nc.scalar.dma_start(
                            table1[gt * P:(gt + nt) * P,
                                   0:cfg.R1USE].rearrange(
                                "(t p) c -> p t c", t=nt), rows[:])
                    r0, r1 = seg * P, (seg + ntile) * P
                    if cfg.DUP and r0 < cfg.DUP:
                        r1 = min(r1, cfg.DUP)
                        nc.scalar.dma_start(
                            table1[NPOS + r0:NPOS + r1, 0:cfg.R1USE],
                            table1[r0:r1, 0:cfg.R1USE])

            # ---------------- Edge phase helper ---------------------------
            def edge_layer(lay, table, DL, RWx, adb, rowlen, postproc,
                           bp, sp, bpp):
                """rowlen in table-dtype elements; payload head-major."""
                tdt = DT2 if lay == 1 else DT8
                Rsh = prep.R1 if lay == 1 else prep.R2
                groups = prep.groups1 if lay == 1 else prep.groups2
                col = {0: 0, 1: 0}
                for gi, g in enumerate(groups):
                    gt = {}
                    for pss in (0, 1):
                        ncols = prep.gcols[(lay, pss)][gi]
                        if ncols == 0:
                            continue
                        n = ncols * P
                        ist = bp.tile([P, n // 16], I16, tag=f"ist{pss}")
                        nc.scalar.dma_start(
                            ist[:], is_d[(lay, pss)][:, col[pss]:col[pss]
                                                     + n // 16])
                        col[pss] += n // 16
                        tbl = (table[0:min(cfg.LO, NPOS), :] if pss == 0
                               else table[cfg.LO:, :])
                        gtile = bp.tile([P, ncols, rowlen], tdt,
                                        tag=f"g{pss}")
                        nc.gpsimd.dma_gather(
                            gtile[:], tbl, ist[:], n, n, rowlen,
                            single_packet=False)
                        gt[pss] = gtile

                    off = {0: 0, 1: 0}
                    for b in g:
                        Rl = int(Rsh[0][b])
                        Rh = int(Rsh[1][b])
                        ps1 = bpp.tile([P, RWx], F32, tag="ps1")
                        first = True
                        for pss, Rn in ((0, Rl), (1, Rh)):
                            if Rn == 0:
                                continue
                            gsl = gt[pss][:, off[pss]:off[pss] + Rn, :]
                            off[pss] += Rn
                            rhs = sp.tile([P, Rn, RWx], DT2, tag="rhs")
                            if lay == 1:
                                asl = gsl[:, :, DL:DL + H]
                            else:
                                asl = gsl[:, :, DL:DL + 2 * H].bitcast(DT2)
                            lg = sp.tile([P, Rn, H], DT2, tag="lg")
                            nc.vector.tensor_tensor(
                                out=lg[:], in0=asl,
                                in1=adb[:, b, None, :].broadcast_to(
                                    [P, Rn, H]),
                                op=ALU.add)
                            lr2 = sp.tile([P, Rn, H], DT2, tag="lr2")
                            nc.vector.tensor_scalar_mul(lr2[:], lg[:], 0.2)
                            nc.vector.tensor_tensor(
                                out=lg[:], in0=lg[:], in1=lr2[:], op=ALU.max)
                            rh4 = rhs[:].rearrange("p r (h c) -> p r h c",
                                                   h=H)
                            nc.scalar.activation(
                                rh4[:, :, :, 0:2],
                                lg[:, :, :, None].broadcast_to(
                                    [P, Rn, H, 2]),
                                ACTF.Exp)
                            g4 = gsl[:, :, 0:DL].rearrange(
                                "p r (h c) -> p r h c", h=H)
                            dph = DL // H
                            nc.vector.tensor_tensor(
                                out=rh4[:, :, :, 2:2 + dph].rearrange(
                                    "p r h (a b) -> p r h a b", b=2),
                                in0=g4[:].rearrange(
                                    "p r h (a b) -> p r h a b", b=2),
                                in1=rh4[:, :, :, 0:2][:, :, :, None, :]
                                .broadcast_to([P, Rn, H, dph // 2, 2]),
                                op=ALU.mult)
                            for r in range(Rn):
                                last = (pss == 1 or Rh == 0) and r == Rn - 1
                                nc.tensor.matmul(
                                    ps1[:], ident_sb[:], rhs[:, r, :],
                                    start=first, stop=last)
                                first = False
                        postproc(b, ps1)

            # ---- Phase B: layer-1 edges + layer-2 dense -------------------
            with tc.tile_pool(name="B", bufs=2) as bp, \
                 tc.tile_pool(name="Bs", bufs=2) as sp, \
                 tc.tile_pool(name="Bp", bufs=3, space="PSUM") as bpp, \
                 tc.tile_pool(name="Bp2", bufs=2, space="PSUM") as bpp2:

                h2acc = {"t": None}

                def post1(b, ps1):
                    p4 = ps1[:].rearrange("p (h c) -> p h c", h=H)
                    dn = sp.tile([P, H], F32, tag="dn")
                    nc.vector.tensor_scalar_add(dn[:], p4[:, :, 0], 1e-16)
                    rc = sp.tile([P, H], F32, tag="rc")
                    nc.vector.reciprocal(rc[:], dn[:])
                    o1 = sp.tile([P, D1], F32, tag="o1")
                    nc.vector.tensor_tensor(
                        out=o1[:].rearrange("p (h d) -> p h d", h=H),
                        in0=p4[:, :, 2:2 + HID],
                        in1=rc[:][:, :, None].broadcast_to([P, H, HID]),
                        op=ALU.mult)
                    # sfull = elu(o1)+1 = min(exp(o1),1) + relu(o1)
                    exf = sp.tile([P, D1], DT2, tag="exf")
                    nc.scalar.activation(exf[:], o1[:], ACTF.Exp)
                    exm = sp.tile([P, D1], DT2, tag="exm")
                    nc.vector.tensor_scalar_min(exm[:], exf[:], 1.0)
                    r1 = sp.tile([P, D1], DT2, tag="r1")
                    nc.scalar.activation(r1[:], o1[:], ACTF.Relu)
                    sfull = sp.tile([P, D1], DT2, tag="sfull")
                    nc.vector.tensor_tensor(
                        out=sfull[:], in0=exm[:], in1=r1[:], op=ALU.add)
                    ps2 = bpp2.tile([P, W2C], F32, tag="ps2")
                    for k in range(K2):
                        pt = bpp2.tile([P, P], DT2, tag="pt")
                        nc.tensor.transpose(
                            pt[:], sfull[:, k * P:(k + 1) * P], ident_sb[:])
                        st = sp.tile([P, P], DT2, tag="st")
                        nc.scalar.copy(st[:], pt[:])
                        nc.tensor.matmul(ps2[:], st[:], w2_sb[k][:],
                                         start=(k == 0), stop=False)
                    nc.tensor.matmul(ps2[:], ones1_sb[:], bias2_sb[:],
                                     start=False, stop=True)
                    # packed h2 row: [h2 fp8 | a_s2 f32->fp16 bitcast]
                    a = int(np.searchsorted(prep.ag_start, b, "right")) - 1
                    pos = b - int(prep.ag_start[a])
                    nbl = prep.ag[a]
                    if pos == 0:
                        h2acc["t"] = bp.tile([P, nbl, cfg.R2USE], DT8,
                                             tag="h2acc", name="h2acc")
                    nc.scalar.copy(h2acc["t"][:, pos, 0:D2], ps2[:, 0:D2])
                    mask_as(h2acc["t"][:, pos, D2:D2 + 2 * H].bitcast(DT2),
                            ps2[:, D2:D2 + H], b)
                    nc.vector.tensor_copy(adb2[:, b, :],
                                          ps2[:, D2 + H:D2 + 2 * H])
                    if pos == nbl - 1:
                        rows_n = nbl * P
                        nc.scalar.dma_start(
                            h2sh[a][0:rows_n, :].rearrange(
                                "(t p) c -> p t c", t=nbl), h2acc["t"][:])
                        gbase = int(prep.ag_base[a])
                        nrow = NCORES * rows_n
                        if collective:
                            nc.gpsimd.collective_compute(
                                "AllGather", ALU.bypass,
                                replica_groups=[list(range(NCORES))],
                                ins=[h2sh[a][0:rows_n, :].opt()],
                                outs=[table2p[gbase:gbase + nrow, :].opt()],
                            )
                        else:
                            for rr in range(NCORES):
                                bs = gbase + rr * rows_n
                                nc.sync.dma_start(
                                    table2p[bs:bs + rows_n, :],
                                    h2sh[a][0:rows_n, :])
                        # expansion runs on the sync queue, which carries
                        # ONLY collective-dependent copies during the edge
                        # phases: its head-of-line wait blocks nothing else
                        nc.sync.dma_start(
                            table2[gbase:gbase + nrow, 0:cfg.R2USE],
                            table2p[gbase:gbase + nrow, :])
                        if cfg.DUP and gbase < cfg.DUP <= gbase + nrow:
                            nc.sync.dma_start(
                                table2[NPOS:NPOS + cfg.DUP, 0:cfg.R2USE],
                                table2p[0:cfg.DUP, :])

                edge_layer(1, table1, D1, RW1, adb1, cfg.ROW1, post1,
                           bp, sp, bpp)

            # --------------- Phase D: layer-2 edges -----------------------
            with tc.tile_pool(name="D", bufs=2) as bp, \
                 tc.tile_pool(name="Ds", bufs=2) as sp, \
                 tc.tile_pool(name="Dp", bufs=3, space="PSUM") as bpp:

                def post2(b, ps1):
                    p4 = ps1[:].rearrange("p (h c) -> p h c", h=H)
                    dn = sp.tile([P, H], F32, tag="dn")
                    nc.vector.tensor_scalar_add(dn[:], p4[:, :, 0], 1e-16)
                    rc = sp.tile([P, H], F32, tag="rc")
                    nc.vector.reciprocal(rc[:], dn[:])
                    o2 = sp.tile([P, D2], F32, tag="o2")
                    nc.vector.tensor_tensor(
                        out=o2[:].rearrange("p (h d) -> p h d", h=H),
                        in0=p4[:, :, 2:2 + LB],
                        in1=rc[:][:, :, None].broadcast_to([P, H, LB]),
                        op=ALU.mult)
                    en = sp.tile([P, D2], F32, tag="en")
                    nc.scalar.activation(en[:], o2[:], ACTF.Exp, scale=-1.0)
                    nc.vector.tensor_scalar_add(en[:], en[:], 1.0)
                    sg = sp.tile([P, D2], F32, tag="sg")
                    nc.vector.reciprocal(sg[:], en[:])
                    nc.scalar.dma_start(out[b * P:(b + 1) * P, :], sg[:])

                edge_layer(2, table2, D2, RW2, adb2, cfg.ROW2, post2,
                           bp, sp, bpp)

    nc.compile()
    return nc


def make_inputs(cfg, prep, x, W1, att_src1, att_dst1, b1, W2, att_src2,
                att_dst2, b2):
    """Per-core in_maps for the SPMD program."""
    H, HID, LB = cfg.HEADS, cfg.HID, cfg.LABELS
    D1, D2 = cfg.D1, cfg.D2
    W1 = np.asarray(W1, np.float32)
    W2 = np.asarray(W2, np.float32)
    as1 = np.asarray(att_src1, np.float32)
    ad1 = np.asarray(att_dst1, np.float32)
    as2 = np.asarray(att_src2, np.float32)
    ad2 = np.asarray(att_dst2, np.float32)
    b1 = np.asarray(b1, np.float32)
    b2 = np.asarray(b2, np.float32)

    # head-major payload: row = [h0 (HID) | h1 | h2 | h3 | a_s]; W columns
    # are already head-major (reshape (H, HID)) so plain concat works.
    A_s1 = np.einsum("ihc,hc->ih", W1.reshape(-1, H, HID), as1)
    A_d1 = np.einsum("ihc,hc->ih", W1.reshape(-1, H, HID), ad1)
    w1e = np.concatenate([W1, A_s1, A_d1], axis=1).astype(np.float16)
    b1h = b1.reshape(H, HID)
    bias1_row = np.concatenate(
        [b1, np.einsum("hc,hc->h", b1h, as1), np.einsum("hc,hc->h", b1h, ad1)]
    ).astype(np.float32)[None, :]

    A_s2 = np.einsum("ihc,hc->ih", W2.reshape(-1, H, LB), as2)
    A_d2 = np.einsum("ihc,hc->ih", W2.reshape(-1, H, LB), ad2)
    w2e_f = np.concatenate([W2, A_s2, A_d2], axis=1)
    b2h = b2.reshape(H, LB)
    bias2_row = (np.concatenate(
        [b2, np.einsum("hc,hc->h", b2h, as2), np.einsum("hc,hc->h", b2h, ad2)])
                 - w2e_f.sum(axis=0)).astype(np.float32)[None, :]
    w2e = w2e_f.astype(np.float16)

    ident = np.eye(P, dtype=np.float16)
    ones1 = np.ones((1, P), np.float32)
    padflag = np.zeros((P, 4), np.float32)
    padflag[:, 0] = 1.0
    padflag[P - 1, 0] = 0.0
    padflag[P - 1, 1] = ASENT
    # tail block: slots [used_tail .. P) unused
    ncount = np.bincount(prep.node_core, minlength=NCORES)
    assert ncount.max() - ncount.min() <= 1
    used_tail = int(ncount.max()) - (P - 1) - (cfg.NB - 2) * P
    padflag[:, 2] = 1.0
    if used_tail < P:
        padflag[used_tail:, 2] = 0.0
        padflag[used_tail:, 3] = ASENT

    # global position-ordered xT (tile-major), then per-core rotation
    x8 = np.asarray(x, np.float32)
    gpos = prep.node_core * cfg.own_pad + prep.rank
    xg = np.zeros((cfg.NPOS, cfg.IN_F), np.float32)
    xg[gpos] = x8
    xTg = np.ascontiguousarray(xg.T).astype(ml_dtypes.float8_e4m3fn)

    in_maps = []
    for c in range(NCORES):
        xTc = np.ascontiguousarray(np.roll(xTg, -c * cfg.own_pad, axis=1))
        m = {
            "xT": xTc,
            "w1e": w1e, "w2e": w2e,
            "bias1": bias1_row, "bias2": bias2_row,
            "ones1": ones1, "ident": ident, "padflag": padflag,
            "dz": np.zeros((P, 8), np.int16),
        }
        for lay in (1, 2):
            for pss in range(2):
                m[f"is{lay}p{pss}"] = prep.idx_t[(lay, c, pss)]
        in_maps.append(m)
    return in_maps, bool(np.any(b1 != 0))


def assemble_output(cfg, prep, results):
    big = np.concatenate([results[c]["out"] for c in range(NCORES)], axis=0)
    gpos = prep.node_core * cfg.own_pad + prep.rank
    return np.ascontiguousarray(big[gpos]).astype(np.float32)


_CACHE = {}


def _get_program(cfg, prep, with_bias1):
    key = (cfg.N, cfg.IN_F, cfg.HEADS, cfg.HID, cfg.LABELS, with_bias1,
           tuple(prep.R1.reshape(-1)), tuple(prep.R2.reshape(-1)))
    if key not in _CACHE:
        _CACHE[key] = build_program(cfg, prep, with_bias1)
    return _CACHE[key]


def kernel(x, edge_index, W1, att_src1, att_dst1, b1, W2, att_src2, att_dst2,
           b2):
    x = np.asarray(x)
    cfg = Cfg(n_nodes=x.shape[0], in_f=x.shape[1],
              hid=np.asarray(att_src1).shape[1],
              heads=np.asarray(att_src1).shape[0],
              labels=np.asarray(att_src2).shape[1])
    prep = HostPrep(cfg, np.asarray(edge_index))
    in_maps, with_bias1 = make_inputs(cfg, prep, x, W1, att_src1, att_dst1,
                                      b1, W2, att_src2, att_dst2, b2)
    nc = _get_program(cfg, prep, with_bias1)
    res = run_bass_kernel_spmd(nc, in_maps, core_ids=list(range(NCORES)))
    return assemble_output(cfg, prep, res.results)


# revision 44
# speedup vs baseline: 1.0186x; 1.0062x over previous
"""GAT (2-layer, 4-head) forward on 8 Trainium2 NeuronCores (Bass/Tile).

v3 design — slot-aligned rounds:
Destination nodes are dealt to 8 cores (degree-balanced snake deal) and,
within each core, lex-sorted by (deg_lo1, deg_hi1, deg_lo2) into blocks of
128 slots. Edge processing is round-based: round r of block b holds, at
partition s, the r-th incoming edge of the node at slot s (sentinel row
pads, whose a_s = -1e4 makes exp() == 0). The baseline's one-hot scatter
matrices disappear: each round accumulates into the block PSUM through an
identity matmul, and a_d lives in SBUF per-block tiles aligned to
partitions (no per-edge a_d gather at all). Rounds split into lo/hi passes
by int16 index reach. Layer-1 table rows are fp16 [4x(64 h) | a_s] 768B,
rotated so own nodes come first; xT ships fp8. Layer-2 rows are fp8
[4x(32 h2) | a_s2 fp16] packed to 136B for a pipelined AllGather in groups
sized [16,16,8,8,1] blocks (the int16 boundary falls exactly between
groups), then locally expanded to 256B-stride gather rows. exp() is
written as adjacent pairs so payload multiplies run in DVE 2x mode.
"""
import sys

sys.path.insert(0, "/opt/trn_rl_repo")

import numpy as np
import ml_dtypes

import concourse.bass as bass
import concourse.mybir as mybir
import concourse.tile as tile
from concourse import bacc
from concourse.bass_utils import run_bass_kernel_spmd

DT2 = mybir.dt.float16
DT8 = mybir.dt.float8e4
F32 = mybir.dt.float32
I16 = mybir.dt.int16
ALU = mybir.AluOpType
ACTF = mybir.ActivationFunctionType

NCORES = 8
P = 128
ASENT = -10000.0
CAPR = 56  # max rounds per gather group (SBUF budget)


class Cfg:
    def __init__(self, n_nodes=50000, in_f=256, hid=64, heads=4, labels=32,
                 lo_limit=32768):
        self.N = n_nodes
        self.IN_F = in_f
        self.HID = hid
        self.HEADS = heads
        self.LABELS = labels
        self.D1 = heads * hid
        self.D2 = heads * labels
        self.LO = lo_limit
        self.own = -(-n_nodes // NCORES)
        # one reserved pad slot (block 0 slot 127) + at least one tail pad
        self.NB = -(-(self.own + 2) // P)
        self.own_pad = self.NB * P
        self.NPOS = NCORES * self.own_pad
        self.NT = self.NPOS // P
        # rows [0, DUP) are duplicated at [NPOS, NPOS+DUP): edges from them
        # may be fetched through either the lo or the hi index window
        self.DUP = max(0, min(self.LO, self.LO + 32768 - self.NPOS,
                              self.NPOS))
        if self.NPOS <= self.LO:
            self.DUP = 0
        self.TROWS = self.NPOS + self.DUP
        # dup region of table2 is written by a second AllGather covering
        # the whole first collective group
        if self.DUP:
            lo_blocks = min(self.LO // (NCORES * P), self.NB)
            self.DUPREG = NCORES * P * min(16, max(lo_blocks, 1))
        else:
            self.DUPREG = 0
        self.TROWS2 = self.NPOS + self.DUPREG
        # table1 fp16 row: [4*(hid) | a_s (H)] ; pitch = 256B multiple
        self.R1USE = self.D1 + heads
        self.ROW1 = -(-(2 * self.R1USE) // 256) * 128  # pitch in fp16 els
        # table2 fp8 row: [4*(labels) | a_s2 as 2 fp8 bytes per head]
        self.R2USE = self.D2 + 2 * heads             # packed bytes
        self.ROW2 = -(-self.R2USE // 256) * 256      # gather pitch (bytes)
        self.W1C = self.D1 + 2 * heads
        self.W2C = self.D2 + 2 * heads
        self.HB1 = 2 + hid
        self.HB2 = 2 + labels
        self.RW1 = heads * self.HB1
        self.RW2 = heads * self.HB2


def _wrap_idx(idx):
    """idx [n] (n%16==0) -> [128, n//16] int16 (16-row wrap, tiled x8)."""
    n = len(idx)
    return np.tile(np.asarray(idx, np.int16).reshape(n // 16, 16).T, (8, 1))


def _ag_sizes(cfg):
    """AllGather group sizes (in blocks). The cumulative row count of the
    groups crosses cfg.LO exactly at a group boundary when possible; the
    final group is a single block so the tail collective is tiny."""
    NB = cfg.NB
    rows_per_blk = NCORES * P
    lo_blocks = min(cfg.LO // rows_per_blk, NB)
    sizes = []

    def chunk(n, maxsz):
        while n > 0:
            t = min(maxsz, n)
            sizes.append(t)
            n -= t

    chunk(lo_blocks, 8)
    rest = NB - lo_blocks
    if rest > 1:
        chunk(rest - 1, 8)
        sizes.append(1)
    elif rest == 1:
        sizes.append(1)
    return sizes


class HostPrep:
    def __init__(self, cfg, edge_index):
        self.cfg = cfg
        N, NB = cfg.N, cfg.NB
        ei = np.asarray(edge_index, np.int64)
        src = np.concatenate([ei[0], np.arange(N, dtype=np.int64)])
        dst = np.concatenate([ei[1], np.arange(N, dtype=np.int64)])
        deg = np.bincount(dst, minlength=N)

        self.ag = np.asarray(_ag_sizes(cfg), np.int64)
        ag_start = np.concatenate([[0], np.cumsum(self.ag)])[:-1]
        self.ag_start = ag_start
        # packed table2 row base of (ag, core): ag_base[a] + c*ag[a]*P
        self.ag_base = np.concatenate(
            [[0], np.cumsum([s * NCORES * P for s in self.ag])])[:-1]
        blk_ag = np.zeros(NB, np.int64)
        for a, s in enumerate(self.ag):
            blk_ag[ag_start[a]:ag_start[a] + s] = a

        # --- phase 1: degree-balanced core deal (snake) -------------------
        order = np.argsort(-deg, kind="stable")
        node_core = np.empty(N, np.int64)
        k = np.arange(N)
        snake = np.where((k // NCORES) % 2 == 0, k % NCORES,
                         NCORES - 1 - (k % NCORES))
        node_core[order] = snake

        def ranks_from_key(key):
            rank = np.empty(N, np.int64)
            for c in range(NCORES):
                nodes = np.where(node_core == c)[0]
                o = np.argsort(key[nodes], kind="stable")
                rr = np.arange(len(nodes))
                # skip reserved pad slot: block0 slot 127
                rr = rr + (rr >= P - 1)
                rank[nodes[o]] = rr
            return rank

        def rows_from_rank(rank):
            csrc = node_core[src]
            cdst = node_core[dst]
            rp1 = ((csrc - cdst) % NCORES) * cfg.own_pad + rank[src]
            blk_s = rank[src] // P
            a_s = blk_ag[blk_s]
            g2r = (self.ag_base[a_s] + csrc * (self.ag[a_s] * P)
                   + (blk_s - ag_start[a_s]) * P + rank[src] % P)
            return rp1, g2r

        def region(r):
            # 0 = flex (duplicated), 1 = lo-fixed, 2 = hi-fixed
            return np.where(r < cfg.DUP, 0, np.where(r < cfg.LO, 1, 2))

        B51 = 64
        rank = ranks_from_key(deg)
        for _ in range(3):
            rp1, g2r = rows_from_rank(rank)
            r1, r2 = region(rp1), region(g2r)
            l1 = np.bincount(dst[r1 == 1], minlength=N).clip(0, B51 - 1)
            h1 = np.bincount(dst[r1 == 2], minlength=N).clip(0, B51 - 1)
            l2 = np.bincount(dst[r2 == 1], minlength=N).clip(0, B51 - 1)
            key = (deg.clip(0, B51 - 1) * B51**3 + l1 * B51**2
                   + h1 * B51 + l2)
            rank = ranks_from_key(-key)
        rp1, g2r = rows_from_rank(rank)

        self.node_core = node_core
        self.rank = rank
        ecore = node_core[dst]
        eblk = rank[dst] // P
        eslot = rank[dst] % P

        # sentinels (block0 slot127 pad; tail pads in last block)
        # L1 (rotated): own b0s127 row=127 (lo); core+k b0s127 first >= LO
        self.sent1 = {0: P - 1, 1: -1}
        for k2 in range(1, NCORES):
            r = k2 * cfg.own_pad + P - 1
            if r >= cfg.LO:
                self.sent1[1] = r - cfg.LO
                break
        # L2 (global packed): core0 b0 s127 (lo); tail pad of last ag (hi)
        self.sent2 = {0: P - 1, 1: -1}
        r = int(self.ag_base[-1] + 0 + (NB - 1 - ag_start[-1]) * P + P - 1)
        if r >= cfg.LO:
            self.sent2[1] = r - cfg.LO
        else:
            self.sent2[1] = self.sent2[0]  # table fits in lo; unused

        # --- rounds structure with flex balancing, shared over cores ------
        def build(rp, lo_assign, sent):
            Rr = np.zeros((NCORES, 2, NB), np.int64)
            okey = (ecore * 2 + (~lo_assign).astype(np.int64)) * NB * P \
                + eblk * P + eslot
            o = np.argsort(okey, kind="stable")
            so = okey[o]
            rv = rp[o]
            runstart = np.r_[True, so[1:] != so[:-1]]
            runid = np.cumsum(runstart) - 1
            first = np.where(runstart)[0]
            rno = np.arange(len(so)) - first[runid]
            kc = so // (2 * NB * P)
            kp = (so // (NB * P)) % 2
            kb = (so // P) % NB
            ks = so % P
            np.maximum.at(Rr, (kc, kp, kb), rno + 1)
            Rsh = Rr.max(axis=0)  # [2, NB] shared across cores
            Rmax = int(Rsh.max()) if Rsh.size else 0
            big = np.full((NCORES, 2, NB, max(Rmax, 1), P), -1, np.int64)
            big[kc, kp, kb, rno, ks] = rv
            idx = {}
            for c in range(NCORES):
                for pss in range(2):
                    for b in range(NB):
                        R = int(Rsh[pss][b])
                        if R:
                            a = big[c, pss, b, :R].copy()
                            a[a < 0] = sent[pss]
                            idx[(c, pss, b)] = a
            return Rsh, idx

        def assign_pass(rp):
            """Per-edge lo/hi assignment: fixed by region, flex balanced
            per (core, block) to minimize Tlo+Thi."""
            reg = region(rp)
            cnt = np.zeros((3, NCORES, NB, P), np.int64)
            np.add.at(cnt, (reg, ecore, eblk, eslot), 1)
            a_of = np.zeros((NCORES, NB, P), np.int64)
            for c in range(NCORES):
                for b in range(NB):
                    f, lmin, h = cnt[0, c, b], cnt[1, c, b], cnt[2, c, b]
                    lo0 = int(lmin.max())
                    hi0 = int(h.max())
                    best = None
                    for Thi in range(hi0, int((h + f).max()) + 1):
                        a_min = np.maximum(h + f - Thi, 0)
                        Tlo = max(lo0, int((lmin + a_min).max()))
                        if best is None or Tlo + Thi < best[0]:
                            best = (Tlo + Thi, np.minimum(a_min, f))
                        if Tlo == lo0:
                            break
                    a_of[c, b] = best[1]
            # per-edge: flex edge j-th of its slot -> lo if j < a_of
            fi = np.where(reg == 0)[0]
            fkey = (ecore[fi] * NB + eblk[fi]) * P + eslot[fi]
            o = np.argsort(fkey, kind="stable")
            so = fkey[o]
            runstart = np.r_[True, so[1:] != so[:-1]] if len(so) \
                else np.zeros(0, bool)
            runid = np.cumsum(runstart) - 1
            first = np.where(runstart)[0]
            jn = np.arange(len(so)) - first[runid] if len(so) \
                else np.zeros(0, np.int64)
            lo_assign = reg == 1
            sel = fi[o]
            lo_assign[sel] = jn < a_of[ecore[sel], eblk[sel], eslot[sel]]
            return lo_assign

        def eff_row(rp, lo_assign):
            """index value: lo window row, or hi window row (dup if flex)."""
            hi_row = np.where(rp < cfg.DUP, rp + cfg.NPOS, rp)
            return np.where(lo_assign, rp, hi_row - cfg.LO)

        la1 = assign_pass(rp1)
        self.R1, idx1 = build(eff_row(rp1, la1), la1, self.sent1)
        la2 = assign_pass(g2r)
        self.R2, idx2 = build(eff_row(g2r, la2), la2, self.sent2)

        # --- gather groups per layer (greedy, within AG boundaries) -------
        def mkgroups(Rsh):
            groups = []
            cur = []
            cur_r = 0
            for b in range(NB):
                rb = int(Rsh[0][b] + Rsh[1][b])
                if cur and (cur_r + rb > CAPR or blk_ag[b] != blk_ag[cur[0]]):
                    groups.append(cur)
                    cur, cur_r = [], 0
                cur.append(b)
                cur_r += rb
            if cur:
                groups.append(cur)
            return groups

        self.groups1 = mkgroups(self.R1)
        self.groups2 = mkgroups(self.R2)

        # --- flattened idx tensors per (core, layer, pass) ----------------
        self.idx_t = {}
        self.gcols = {}
        for lay, (Rsh, idx, groups) in (
                (1, (self.R1, idx1, self.groups1)),
                (2, (self.R2, idx2, self.groups2))):
            for pss in range(2):
                gc = []
                for g in groups:
                    gc.append(int(sum(Rsh[pss][b] for b in g)))
                self.gcols[(lay, pss)] = gc
            for c in range(NCORES):
                for pss in range(2):
                    cols = []
                    for g in groups:
                        for b in g:
                            if Rsh[pss][b]:
                                cols.append(idx[(c, pss, b)].reshape(-1))
                    flat = (np.concatenate(cols) if cols
                            else np.zeros(0, np.int64))
                    if len(flat) == 0:
                        flat = np.full(16, 0, np.int64)
                    self.idx_t[(lay, c, pss)] = _wrap_idx(flat)

        tot_rounds = int(self.R1.sum() + self.R2.sum()) * NCORES
        self.pad_frac = tot_rounds * P / (2 * len(src)) - 1.0


def build_program(cfg, prep, with_bias1, collective=True):
    nc = bacc.Bacc("TRN2", target_bir_lowering=False, debug=False,
                   num_devices=NCORES)
    H = cfg.HEADS
    D1, D2 = cfg.D1, cfg.D2
    NB, NPOS, NT = cfg.NB, cfg.NPOS, cfg.NT
    HID, LB = cfg.HID, cfg.LABELS
    RW1, RW2 = cfg.RW1, cfg.RW2
    K1 = cfg.IN_F // P
    K2 = D1 // P
    W1C, W2C = cfg.W1C, cfg.W2C
    NAG = len(prep.ag)

    n_idx = {}
    for lay in (1, 2):
        for pss in range(2):
            n_idx[(lay, pss)] = prep.idx_t[(lay, 0, pss)].shape[1] * 16

    xT = nc.dram_tensor("xT", [cfg.IN_F, NPOS], DT8, kind="ExternalInput")
    w1e = nc.dram_tensor("w1e", [cfg.IN_F, W1C], DT2, kind="ExternalInput")
    w2e = nc.dram_tensor("w2e", [D1, W2C], DT2, kind="ExternalInput")
    bias1 = nc.dram_tensor("bias1", [1, W1C], F32, kind="ExternalInput")
    bias2 = nc.dram_tensor("bias2", [1, W2C], F32, kind="ExternalInput")
    ones1 = nc.dram_tensor("ones1", [1, P], F32, kind="ExternalInput")
    ident = nc.dram_tensor("ident", [P, P], DT2, kind="ExternalInput")
    # [flag_b0, offs_b0, flag_tail, offs_tail]
    padflag = nc.dram_tensor("padflag", [P, 4], F32, kind="ExternalInput")
    dz = nc.dram_tensor("dz", [P, 8], I16, kind="ExternalInput")
    is_d = {}
    for lay in (1, 2):
        for pss in range(2):
            is_d[(lay, pss)] = nc.dram_tensor(
                f"is{lay}p{pss}", [P, max(n_idx[(lay, pss)] // 16, 16)],
                I16, kind="ExternalInput")
    out = nc.dram_tensor("out", [cfg.own_pad, D2], F32, kind="ExternalOutput")

    with tile.TileContext(nc) as tc:
        with tc.tile_pool(name="dram", bufs=1, space="DRAM") as dram, \
             tc.tile_pool(name="const", bufs=1) as cp:
            table1 = dram.tile([cfg.TROWS, cfg.ROW1], DT2)
            h2sh = [dram.tile([prep.ag[a] * P, cfg.R2USE], DT8,
                              tag=f"h2sh{a}", name=f"h2sh{a}")
                    for a in range(NAG)]
            table2p = dram.tile([NPOS, cfg.R2USE], DT8)
            table2 = dram.tile([cfg.TROWS, cfg.ROW2], DT8)

            def load_const(name, dram_t, shape, dt):
                t = cp.tile(shape, dt, tag=name, name=name + "_sb")
                nc.sync.dma_start(t[:], dram_t[:])
                return t

            # warm-up gather hoists the gpsimd library load to t~0
            dz_sb = load_const("dz", dz, [P, 8], I16)
            warm = cp.tile([P, 1, cfg.ROW1], DT2, tag="warm", name="warm")
            nc.gpsimd.dma_gather(warm[:], table1[:], dz_sb[:], P, P,
                                 cfg.ROW1, single_packet=False)
            ident_sb = load_const("ident", ident, [P, P], DT2)
            bias1_sb = load_const("bias1", bias1, [1, W1C], F32)
            bias2_sb = load_const("bias2", bias2, [1, W2C], F32)
            ones1_sb = load_const("ones1", ones1, [1, P], F32)
            pf_sb = load_const("padflag", padflag, [P, 4], F32)
            w1_sb = [cp.tile([P, W1C], DT2, tag=f"w1_{k}", name=f"w1sb{k}")
                     for k in range(K1)]
            for k in range(K1):
                nc.sync.dma_start(w1_sb[k][:], w1e[k * P:(k + 1) * P, :])
            w2_sb = [cp.tile([P, W2C], DT2, tag=f"w2_{k}", name=f"w2sb{k}")
                     for k in range(K2)]
            for k in range(K2):
                nc.sync.dma_start(w2_sb[k][:], w2e[k * P:(k + 1) * P, :])
            adb1 = cp.tile([P, NB, H], DT2, tag="adb1", name="adb1")
            adb2 = cp.tile([P, NB, H], DT2, tag="adb2", name="adb2")

            def mask_as(out_ap, in_ap, b, eng=None):
                """write a_s, overwriting pad slots with ASENT."""
                eng = eng or nc.vector
                if b == 0:
                    eng.tensor_scalar(out_ap, in_ap, pf_sb[:, 0:1],
                                      pf_sb[:, 1:2], ALU.mult, ALU.add)
                elif b == NB - 1:
                    eng.tensor_scalar(out_ap, in_ap, pf_sb[:, 2:3],
                                      pf_sb[:, 3:4], ALU.mult, ALU.add)
                else:
                    eng.tensor_copy(out_ap, in_ap)

            # ---------------- Phase A: dense layer 1 (replicated) ---------
            SEG = 48
            with tc.tile_pool(name="dA", bufs=3) as dp, \
                 tc.tile_pool(name="dAp", bufs=6, space="PSUM") as dpp:
                RB = 16
                for seg in range(0, NT, SEG):
                    ntile = min(SEG, NT - seg)
                    xs = [dp.tile([P, ntile * P], DT8, tag=f"xs{k}",
                                  name=f"xs{k}") for k in range(K1)]
                    for k in range(K1):
                        nc.sync.dma_start(
                            xs[k][:],
                            xT[k * P:(k + 1) * P, seg * P:(seg + ntile) * P])
                    for t0 in range(0, ntile, RB):
                        nt = min(RB, ntile - t0)
                        rows = dp.tile([P, nt, cfg.R1USE], DT2, tag="rows")
                        for t in range(t0, t0 + nt):
                            rt = seg + t
                            ps = dpp.tile([P, W1C], F32, tag="ps")
                            for k in range(K1):
                                nc.tensor.matmul(
                                    ps[:], xs[k][:, t * P:(t + 1) * P],
                                    w1_sb[k][:], start=(k == 0),
                                    stop=(k == K1 - 1 and not with_bias1))
                            if with_bias1:
                                nc.tensor.matmul(ps[:], ones1_sb[:],
                                                 bias1_sb[:], start=False,
                                                 stop=True)
                            j = t - t0
                            bpos = rt % NB
                            ncp = D1 if bpos in (0, NB - 1) else D1 + H
                            if rt % 5 < 2:
                                nc.vector.tensor_copy(rows[:, j, 0:ncp],
                                                      ps[:, 0:ncp])
                            else:
                                nc.scalar.copy(rows[:, j, 0:ncp],
                                               ps[:, 0:ncp])
                            if ncp == D1:
                                mask_as(rows[:, j, D1:D1 + H],
                                        ps[:, D1:D1 + H], bpos)
                            if rt < NB:
                                nc.vector.tensor_copy(
                                    adb1[:, rt, :], ps[:, D1 + H:D1 + 2 * H])
                        gt = seg + t0
                        nc.scalar.dma_start(
                            table1[gt * P:(gt + nt) * P,
                                   0:cfg.R1USE].rearrange(
                                "(t p) c -> p t c", t=nt), rows[:])
                    r0, r1 = seg * P, (seg + ntile) * P
                    if cfg.DUP and r0 < cfg.DUP:
                        r1 = min(r1, cfg.DUP)
                        nc.scalar.dma_start(
                            table1[NPOS + r0:NPOS + r1, 0:cfg.R1USE],
                            table1[r0:r1, 0:cfg.R1USE])

            # ---------------- Edge phase helper ---------------------------
            def edge_layer(lay, table, DL, RWx, adb, rowlen, postproc,
                           bp, sp, bpp):
                """rowlen in table-dtype elements; payload head-major."""
                tdt = DT2 if lay == 1 else DT8
                Rsh = prep.R1 if lay == 1 else prep.R2
                groups = prep.groups1 if lay == 1 else prep.groups2
                col = {0: 0, 1: 0}
                for gi, g in enumerate(groups):
                    gt = {}
                    for pss in (0, 1):
                        ncols = prep.gcols[(lay, pss)][gi]
                        if ncols == 0:
                            continue
                        n = ncols * P
                        ist = bp.tile([P, n // 16], I16, tag=f"ist{pss}")
                        nc.scalar.dma_start(
                            ist[:], is_d[(lay, pss)][:, col[pss]:col[pss]
                                                     + n // 16])
                        col[pss] += n // 16
                        tbl = (table[0:min(cfg.LO, NPOS), :] if pss == 0
                               else table[cfg.LO:, :])
                        gtile = bp.tile([P, ncols, rowlen], tdt,
                                        tag=f"g{pss}")
                        nc.gpsimd.dma_gather(
                            gtile[:], tbl, ist[:], n, n, rowlen,
                            single_packet=False)
                        gt[pss] = gtile

                    off = {0: 0, 1: 0}
                    for b in g:
                        Rl = int(Rsh[0][b])
                        Rh = int(Rsh[1][b])
                        ps1 = bpp.tile([P, RWx], F32, tag="ps1")
                        first = True
                        for pss, Rn in ((0, Rl), (1, Rh)):
                            if Rn == 0:
                                continue
                            gsl = gt[pss][:, off[pss]:off[pss] + Rn, :]
                            off[pss] += Rn
                            rhs = sp.tile([P, Rn, RWx], DT2, tag="rhs")
                            if lay == 1:
                                asl = gsl[:, :, DL:DL + H]
                            else:
                                asl = gsl[:, :, DL:DL + 2 * H].bitcast(DT2)
                            lg = sp.tile([P, Rn, H], DT2, tag="lg")
                            nc.vector.tensor_tensor(
                                out=lg[:], in0=asl,
                                in1=adb[:, b, None, :].broadcast_to(
                                    [P, Rn, H]),
                                op=ALU.add)
                            lr2 = sp.tile([P, Rn, H], DT2, tag="lr2")
                            nc.vector.tensor_scalar_mul(lr2[:], lg[:], 0.2)
                            nc.vector.tensor_tensor(
                                out=lg[:], in0=lg[:], in1=lr2[:], op=ALU.max)
                            rh4 = rhs[:].rearrange("p r (h c) -> p r h c",
                                                   h=H)
                            nc.scalar.activation(
                                rh4[:, :, :, 0:2],
                                lg[:, :, :, None].broadcast_to(
                                    [P, Rn, H, 2]),
                                ACTF.Exp)
                            g4 = gsl[:, :, 0:DL].rearrange(
                                "p r (h c) -> p r h c", h=H)
                            dph = DL // H
                            nc.vector.tensor_tensor(
                                out=rh4[:, :, :, 2:2 + dph].rearrange(
                                    "p r h (a b) -> p r h a b", b=2),
                                in0=g4[:].rearrange(
                                    "p r h (a b) -> p r h a b", b=2),
                                in1=rh4[:, :, :, 0:2][:, :, :, None, :]
                                .broadcast_to([P, Rn, H, dph // 2, 2]),
                                op=ALU.mult)
                            for r in range(Rn):
                                last = (pss == 1 or Rh == 0) and r == Rn - 1
                                nc.tensor.matmul(
                                    ps1[:], ident_sb[:], rhs[:, r, :],
                                    start=first, stop=last)
                                first = False
                        postproc(b, ps1)

            # ---- Phase B: layer-1 edges + layer-2 dense -------------------
            with tc.tile_pool(name="B", bufs=2) as bp, \
                 tc.tile_pool(name="Bs", bufs=2) as sp, \
                 tc.tile_pool(name="Bp", bufs=3, space="PSUM") as bpp, \
                 tc.tile_pool(name="Bp2", bufs=2, space="PSUM") as bpp2:

                h2acc = {"t": None}

                def post1(b, ps1):
                    p4 = ps1[:].rearrange("p (h c) -> p h c", h=H)
                    dn = sp.tile([P, H], F32, tag="dn")
                    nc.vector.tensor_scalar_add(dn[:], p4[:, :, 0], 1e-16)
                    rc = sp.tile([P, H], F32, tag="rc")
                    nc.vector.reciprocal(rc[:], dn[:])
                    o1 = sp.tile([P, D1], F32, tag="o1")
                    nc.vector.tensor_tensor(
                        out=o1[:].rearrange("p (h d) -> p h d", h=H),
                        in0=p4[:, :, 2:2 + HID],
                        in1=rc[:][:, :, None].broadcast_to([P, H, HID]),
                        op=ALU.mult)
                    # sfull = elu(o1)+1 = min(exp(o1),1) + relu(o1)
                    exf = sp.tile([P, D1], DT2, tag="exf")
                    nc.scalar.activation(exf[:], o1[:], ACTF.Exp)
                    exm = sp.tile([P, D1], DT2, tag="exm")
                    nc.vector.tensor_scalar_min(exm[:], exf[:], 1.0)
                    r1 = sp.tile([P, D1], DT2, tag="r1")
                    nc.scalar.activation(r1[:], o1[:], ACTF.Relu)
                    sfull = sp.tile([P, D1], DT2, tag="sfull")
                    nc.vector.tensor_tensor(
                        out=sfull[:], in0=exm[:], in1=r1[:], op=ALU.add)
                    ps2 = bpp2.tile([P, W2C], F32, tag="ps2")
                    for k in range(K2):
                        pt = bpp2.tile([P, P], DT2, tag="pt")
                        nc.tensor.transpose(
                            pt[:], sfull[:, k * P:(k + 1) * P], ident_sb[:])
                        st = sp.tile([P, P], DT2, tag="st")
                        nc.scalar.copy(st[:], pt[:])
                        nc.tensor.matmul(ps2[:], st[:], w2_sb[k][:],
                                         start=(k == 0), stop=False)
                    nc.tensor.matmul(ps2[:], ones1_sb[:], bias2_sb[:],
                                     start=False, stop=True)
                    # packed h2 row: [h2 fp8 | a_s2 f32->fp16 bitcast]
                    a = int(np.searchsorted(prep.ag_start, b, "right")) - 1
                    pos = b - int(prep.ag_start[a])
                    nbl = prep.ag[a]
                    if pos == 0:
                        h2acc["t"] = bp.tile([P, nbl, cfg.R2USE], DT8,
                                             tag="h2acc", name="h2acc")
                    nc.scalar.copy(h2acc["t"][:, pos, 0:D2], ps2[:, 0:D2])
                    mask_as(h2acc["t"][:, pos, D2:D2 + 2 * H].bitcast(DT2),
                            ps2[:, D2:D2 + H], b)
                    nc.vector.tensor_copy(adb2[:, b, :],
                                          ps2[:, D2 + H:D2 + 2 * H])
                    if pos == nbl - 1:
                        rows_n = nbl * P
                        nc.scalar.dma_start(
                            h2sh[a][0:rows_n, :].rearrange(
                                "(t p) c -> p t c", t=nbl), h2acc["t"][:])
                        gbase = int(prep.ag_base[a])
                        nrow = NCORES * rows_n
                        if collective:
                            nc.gpsimd.collective_compute(
                                "AllGather", ALU.bypass,
                                replica_groups=[list(range(NCORES))],
                                ins=[h2sh[a][0:rows_n, :].opt()],
                                outs=[table2p[gbase:gbase + nrow, :].opt()],
                            )
                        else:
                            for rr in range(NCORES):
                                bs = gbase + rr * rows_n
                                nc.sync.dma_start(
                                    table2p[bs:bs + rows_n, :],
                                    h2sh[a][0:rows_n, :])
                        # expansion runs on the sync queue, which carries
                        # ONLY collective-dependent copies during the edge
                        # phases: its head-of-line wait blocks nothing else
                        nc.sync.dma_start(
                            table2[gbase:gbase + nrow, 0:cfg.R2USE],
                            table2p[gbase:gbase + nrow, :])
                        if cfg.DUP and gbase < cfg.DUP <= gbase + nrow:
                            nc.sync.dma_start(
                                table2[NPOS:NPOS + cfg.DUP, 0:cfg.R2USE],
                                table2p[0:cfg.DUP, :])

                edge_layer(1, table1, D1, RW1, adb1, cfg.ROW1, post1,
                           bp, sp, bpp)

            # --------------- Phase D: layer-2 edges -----------------------
            with tc.tile_pool(name="D", bufs=2) as bp, \
                 tc.tile_pool(name="Ds", bufs=2) as sp, \
                 tc.tile_pool(name="Dp", bufs=3, space="PSUM") as bpp:

                def post2(b, ps1):
                    p4 = ps1[:].rearrange("p (h c) -> p h c", h=H)
                    dn = sp.tile([P, H], F32, tag="dn")
                    nc.vector.tensor_scalar_add(dn[:], p4[:, :, 0], 1e-16)
                    rc = sp.tile([P, H], F32, tag="rc")
                    nc.vector.reciprocal(rc[:], dn[:])
                    o2 = sp.tile([P, D2], F32, tag="o2")
                    nc.vector.tensor_tensor(
                        out=o2[:].rearrange("p (h d) -> p h d", h=H),
                        in0=p4[:, :, 2:2 + LB],
                        in1=rc[:][:, :, None].broadcast_to([P, H, LB]),
                        op=ALU.mult)
                    en = sp.tile([P, D2], F32, tag="en")
                    nc.scalar.activation(en[:], o2[:], ACTF.Exp, scale=-1.0)
                    nc.vector.tensor_scalar_add(en[:], en[:], 1.0)
                    sg = sp.tile([P, D2], F32, tag="sg")
                    nc.vector.reciprocal(sg[:], en[:])
                    nc.scalar.dma_start(out[b * P:(b + 1) * P, :], sg[:])

                edge_layer(2, table2, D2, RW2, adb2, cfg.ROW2, post2,
                           bp, sp, bpp)

    nc.compile()
    return nc


def make_inputs(cfg, prep, x, W1, att_src1, att_dst1, b1, W2, att_src2,
                att_dst2, b2):
    """Per-core in_maps for the SPMD program."""
    H, HID, LB = cfg.HEADS, cfg.HID, cfg.LABELS
    D1, D2 = cfg.D1, cfg.D2
    W1 = np.asarray(W1, np.float32)
    W2 = np.asarray(W2, np.float32)
    as1 = np.asarray(att_src1, np.float32)
    ad1 = np.asarray(att_dst1, np.float32)
    as2 = np.asarray(att_src2, np.float32)
    ad2 = np.asarray(att_dst2, np.float32)
    b1 = np.asarray(b1, np.float32)
    b2 = np.asarray(b2, np.float32)

    # head-major payload: row = [h0 (HID) | h1 | h2 | h3 | a_s]; W columns
    # are already head-major (reshape (H, HID)) so plain concat works.
    A_s1 = np.einsum("ihc,hc->ih", W1.reshape(-1, H, HID), as1)
    A_d1 = np.einsum("ihc,hc->ih", W1.reshape(-1, H, HID), ad1)
    w1e = np.concatenate([W1, A_s1, A_d1], axis=1).astype(np.float16)
    b1h = b1.reshape(H, HID)
    bias1_row = np.concatenate(
        [b1, np.einsum("hc,hc->h", b1h, as1), np.einsum("hc,hc->h", b1h, ad1)]
    ).astype(np.float32)[None, :]

    A_s2 = np.einsum("ihc,hc->ih", W2.reshape(-1, H, LB), as2)
    A_d2 = np.einsum("ihc,hc->ih", W2.reshape(-1, H, LB), ad2)
    w2e_f = np.concatenate([W2, A_s2, A_d2], axis=1)
    b2h = b2.reshape(H, LB)
    bias2_row = (np.concatenate(
        [b2, np.einsum("hc,hc->h", b2h, as2), np.einsum("hc,hc->h", b2h, ad2)])
                 - w2e_f.sum(axis=0)).astype(np.float32)[None, :]
    w2e = w2e_f.astype(np.float16)

    ident = np.eye(P, dtype=np.float16)
    ones1 = np.ones((1, P), np.float32)
    padflag = np.zeros((P, 4), np.float32)
    padflag[:, 0] = 1.0
    padflag[P - 1, 0] = 0.0
    padflag[P - 1, 1] = ASENT
    # tail block: slots [used_tail .. P) unused
    ncount = np.bincount(prep.node_core, minlength=NCORES)
    assert ncount.max() - ncount.min() <= 1
    used_tail = int(ncount.max()) - (P - 1) - (cfg.NB - 2) * P
    padflag[:, 2] = 1.0
    if used_tail < P:
        padflag[used_tail:, 2] = 0.0
        padflag[used_tail:, 3] = ASENT

    # global position-ordered xT (tile-major), then per-core rotation
    x8 = np.asarray(x, np.float32)
    gpos = prep.node_core * cfg.own_pad + prep.rank
    xg = np.zeros((cfg.NPOS, cfg.IN_F), np.float32)
    xg[gpos] = x8
    xTg = np.ascontiguousarray(xg.T).astype(ml_dtypes.float8_e4m3fn)

    in_maps = []
    for c in range(NCORES):
        xTc = np.ascontiguousarray(np.roll(xTg, -c * cfg.own_pad, axis=1))
        m = {
            "xT": xTc,
            "w1e": w1e, "w2e": w2e,
            "bias1": bias1_row, "bias2": bias2_row,
            "ones1": ones1, "ident": ident, "padflag": padflag,
            "dz": np.zeros((P, 8), np.int16),
        }
        for lay in (1, 2):
            for pss in range(2):
                m[f"is{lay}p{pss}"] = prep.idx_t[(lay, c, pss)]
        in_maps.append(m)
    return in_maps, bool(np.any(b1 != 0))


def assemble_output(cfg, prep, results):
    big = np.concatenate([results[c]["out"] for c in range(NCORES)], axis=0)
    gpos = prep.node_core * cfg.own_pad + prep.rank
    return np.ascontiguousarray(big[gpos]).astype(np.float32)


_CACHE = {}


def _get_program(cfg, prep, with_bias1):
    key = (cfg.N, cfg.IN_F, cfg.HEADS, cfg.HID, cfg.LABELS, with_bias1,
           tuple(prep.R1.reshape(-1)), tuple(prep.R2.reshape(-1)))
    if key not in _CACHE:
        _CACHE[key] = build_program(cfg, prep, with_bias1)
    return _CACHE[key]


def kernel(x, edge_index, W1, att_src1, att_dst1, b1, W2, att_src2, att_dst2,
           b2):
    x = np.asarray(x)
    cfg = Cfg(n_nodes=x.shape[0], in_f=x.shape[1],
              hid=np.asarray(att_src1).shape[1],
              heads=np.asarray(att_src1).shape[0],
              labels=np.asarray(att_src2).shape[1])
    prep = HostPrep(cfg, np.asarray(edge_index))
    in_maps, with_bias1 = make_inputs(cfg, prep, x, W1, att_src1, att_dst1,
                                      b1, W2, att_src2, att_dst2, b2)
    nc = _get_program(cfg, prep, with_bias1)
    res = run_bass_kernel_spmd(nc, in_maps, core_ids=list(range(NCORES)))
    return assemble_output(cfg, prep, res.results)
